# revision 1
# baseline (speedup 1.0000x reference)
"""nn_CRFLayer: CRF Viterbi decode on 8 Trainium2 NeuronCores.

Data parallel over batch: each core decodes 64 of the 512 sequences.
Self-contained: hardcodes B=512, T=512, D=48, n_cores=8.

Per-core kernel (Tile-scheduled, VectorE + Pool engines):
  Layout: partitions = (b, half) interleaved (p = 2b+ch); each forward step
  processes scores[128, 24 cur, 48 prev]; alpha is stored rotated per
  partition-half with constants pre-rotated to compensate.

  Forward step: Pool computes scores = trans_rep + alpha (the only 2-input
  arith op the Pool engine supports); VectorE does the segmented reduce_max,
  the alpha update (add emit, copy_predicated on the t<len mask, pair-swap
  stream_shuffle), and an exact first-index argmax (is_ge mask -> bf16
  mult by (prev-64) -> segmented reduce_min) that is software-pipelined two
  steps behind the alpha chain so the in-order VectorE stream never stalls
  on Pool latency. Backpointers (bf16) stream to DRAM.

  Backward: one-hot dot-product chain (scalar_tensor_tensor accumulate,
  last_tag injection at t == L-1 via precomputed masks, is_equal one-hot
  regeneration), then mask t >= L to 0 and cast to int32.

All value-producing float ops are the same single fp32 adds as the
reference, so the decoded tags match bitwise-exactly.
"""


import numpy as np
import ml_dtypes

import concourse.bass as bass
import concourse.mybir as mybir
from concourse.tile import TileContext
from concourse.tile_rust import add_dep_helper

AL = mybir.AluOpType
F32 = mybir.dt.float32
BF16 = mybir.dt.bfloat16
I32 = mybir.dt.int32

D = 48
HALF = 24
BLOC = 64
BIG = 64.0

PAIR_SWAP_MASK = [i ^ 1 for i in range(32)]


def make_consts(trans: np.ndarray, T: int) -> dict[str, np.ndarray]:
    """Host-prepared constant tensors (all tiny; derived from trans + shapes)."""
    trans = np.asarray(trans, dtype=np.float32)
    trans_rep = np.zeros((128, HALF, D), dtype=np.float32)
    iota_neg = np.zeros((128, HALF, D), dtype=np.float32)
    for ch in range(2):
        prev = (np.arange(D) + HALF * ch) % D
        cur = HALF * ch + np.arange(HALF)
        block = trans[prev][:, cur].T  # [c24, j]
        for b in range(BLOC):
            p = 2 * b + ch
            trans_rep[p] = block
            iota_neg[p] = prev[None, :] - BIG
    iota_t = np.broadcast_to(np.arange(T, dtype=np.float32)[None, :], (128, T)).copy()
    iota48_nat = np.broadcast_to(np.arange(D, dtype=np.float32)[None, :], (BLOC, D)).copy()
    iota_big_nat = iota48_nat + BIG
    return {
        "trans_rep": trans_rep.reshape(128, HALF * D),
        "iota_neg": iota_neg.reshape(128, HALF * D).astype(ml_dtypes.bfloat16),
        "iota_t_il": iota_t,                      # [128, T] f32 (rows 0..127)
        "iota_t_nat": iota_t[:BLOC].copy(),       # [64, T] f32
        "iota48_nat": iota48_nat,                 # [64, 48] f32
        "iota_big_nat": iota_big_nat,             # [64, 48] f32
    }


def make_core_inputs(logits_core, sent_lengths_core, consts) -> dict[str, np.ndarray]:
    L = np.asarray(sent_lengths_core, dtype=np.float32)
    lg = np.asarray(logits_core, dtype=np.float32)
    T = lg.shape[1]
    lg_il = lg.reshape(BLOC, T, 2, HALF).transpose(0, 2, 1, 3).reshape(128, T, HALF)
    return dict(
        consts,
        logits_il=np.ascontiguousarray(lg_il),
        l_il=np.repeat(L, 2).reshape(128, 1),
        l_nat=L.reshape(BLOC, 1),
    )


def crf_kernel(tc: TileContext, outs, ins, T: int = 512, CK: int = 16, CKB: int = 32, repeat: int = 1):
    nc = tc.nc
    logits_il = ins["logits_il"]      # [128, T, 24] dram f32 (p = 2b+h interleaved)
    tags_out = outs["tags"]           # [64, T] dram i32

    bp_dram_il = nc.dram_tensor("bp_scratch", [128, T, HALF], BF16, kind="Internal").ap()
    alpha_dram = nc.dram_tensor("alpha_scratch", [BLOC, 2, D], F32, kind="Internal").ap()

    import contextlib
    with (
        tc.tile_pool(name="persist", bufs=1) as pp,
        tc.tile_pool(name="chunks", bufs=3) as cp,
        tc.tile_pool(name="work", bufs=4) as wp,
        tc.For_i(0, repeat, 1) if repeat > 1 else contextlib.nullcontext(),
    ):
        # ---- persistent constants ----
        trans_rep = pp.tile([128, HALF, D], F32, tag="trans_rep")
        nc.sync.dma_start(trans_rep[:].rearrange("p a b -> p (a b)"), ins["trans_rep"])
        iota_neg = pp.tile([128, HALF, D], BF16, tag="iota_neg")
        nc.sync.dma_start(iota_neg[:].rearrange("p a b -> p (a b)"), ins["iota_neg"])
        iota48_nat = pp.tile([BLOC, D], F32, tag="iota48_nat")
        nc.sync.dma_start(iota48_nat[:], ins["iota48_nat"])
        iota_big_nat = pp.tile([BLOC, D], F32, tag="iota_big_nat")
        nc.sync.dma_start(iota_big_nat[:], ins["iota_big_nat"])
        l_il = pp.tile([128, 1], F32, tag="l_il")
        nc.sync.dma_start(l_il[:], ins["l_il"])
        l_nat = pp.tile([BLOC, 1], F32, tag="l_nat")
        nc.sync.dma_start(l_nat[:], ins["l_nat"])
        iota_t_il = pp.tile([128, T], F32, tag="iota_t_il")
        nc.sync.dma_start(iota_t_il[:], ins["iota_t_il"])
        iota_t_nat = pp.tile([BLOC, T], F32, tag="iota_t_nat")
        nc.sync.dma_start(iota_t_nat[:], ins["iota_t_nat"])

        # ---- derived masks ----
        valid_il = pp.tile([128, T], mybir.dt.uint8, tag="valid_il")  # t < L[b]
        nc.vector.tensor_scalar(
            out=valid_il[:], in0=iota_t_il[:], scalar1=l_il[:, 0:1], scalar2=None,
            op0=AL.is_lt,
        )
        # ---- forward state ----
        alpha = pp.tile([128, D], F32, tag="alpha")            # [own(24) | other(24)]
        nc.sync.dma_start(alpha[:, 0:HALF], logits_il[:, 0, :])
        nc.vector.stream_shuffle(alpha[:, HALF:D], alpha[:, 0:HALF], mask=PAIR_SWAP_MASK)

        # ---- forward scan ----
        # bp extraction (P5) lags LAG steps behind so the in-order DVE stream
        # never stalls on the Pool-engine P4 latency; each chunk's bp DMA is
        # emitted only after its last P5.
        LAG = 2
        pend = []            # [(f_tile, bp_ch_tile, slot, chunk_id), ...]
        chunk_left = {}      # chunk_id -> outstanding P5 count
        chunk_dma = {}       # chunk_id -> (bp_ch_tile, lo, hi)

        last_sh = [None]

        def emit_p5(entry):
            pf, pbp, pk, cid = entry
            p5i = nc.vector.tensor_reduce(
                out=pbp[:, pk, :], in_=pf[:], axis=mybir.AxisListType.X, op=AL.min,
            )
            if last_sh[0] is not None:
                add_dep_helper(p5i.ins, last_sh[0].ins, sync=False,
                               reason="keep P5 behind the alpha chain")
            chunk_left[cid] -= 1
            if chunk_left[cid] == 0:
                pbp2, lo, hi = chunk_dma[cid]
                nc.sync.dma_start(
                    bp_dram_il[:, lo:hi, :], pbp2[:, lo - cid:hi - cid, :],
                )

        for t0 in range(0, T, CK):
            ck = min(CK, T - t0)
            emit_ch = cp.tile([128, CK, HALF], F32, tag="emit_ch")
            nc.sync.dma_start(emit_ch[:, 0:ck, :], logits_il[:, t0:t0 + ck, :])
            bp_ch = cp.tile([128, CK, HALF], BF16, tag="bp_ch")
            cid = t0
            lo = max(t0, 1)
            chunk_left[cid] = (t0 + ck) - lo
            chunk_dma[cid] = (bp_ch, lo, t0 + ck)
            for t in range(lo, t0 + ck):
                k = t - t0
                scores = wp.tile([128, HALF, D], F32, tag="scores")
                maxv = wp.tile([128, HALF], F32, tag="maxv")
                mask = wp.tile([128, HALF, D], BF16, tag="mask")
                f = wp.tile([128, HALF, D], BF16, tag="f")
                u = wp.tile([128, HALF], F32, tag="u")

                alpha_b = alpha[:].unsqueeze(1).broadcast_to([128, HALF, D])
                nc.gpsimd.tensor_tensor(out=scores[:], in0=trans_rep[:], in1=alpha_b, op=AL.add)
                nc.vector.tensor_reduce(
                    out=maxv[:], in_=scores[:], axis=mybir.AxisListType.X, op=AL.max,
                )
                # alpha update first (keeps the cross-step chain short)
                nc.vector.tensor_add(out=u[:], in0=maxv[:], in1=emit_ch[:, k, :])
                valid_b = valid_il[:, t:t + 1].broadcast_to([128, HALF])
                nc.vector.copy_predicated(out=alpha[:, 0:HALF], mask=valid_b, data=u[:])
                last_sh[0] = nc.vector.stream_shuffle(
                    alpha[:, HALF:D], alpha[:, 0:HALF], mask=PAIR_SWAP_MASK,
                )
                # bp extraction fills the DVE window under the next Pool P1
                while len(pend) >= LAG:
                    emit_p5(pend.pop(0))
                maxv_b = maxv[:].unsqueeze(2).broadcast_to([128, HALF, D])
                p3i = nc.vector.tensor_tensor(
                    out=mask[:], in0=scores[:], in1=maxv_b, op=AL.is_ge,
                )
                add_dep_helper(p3i.ins, last_sh[0].ins, sync=False,
                               reason="keep P3 behind the alpha chain")
                nc.vector.tensor_tensor(
                    out=f[:], in0=mask[:], in1=iota_neg[:], op=AL.mult,
                )
                pend.append((f, bp_ch, k, cid))
        while pend:
            emit_p5(pend.pop(0))

        # ---- last_tag from final alpha ----
        # write alpha (even partitions hold natural alpha[b, 0:48]) to DRAM, read back
        nc.sync.dma_start(alpha_dram.rearrange("b h d -> (b h) d"), alpha[:])
        alpha_nat = pp.tile([BLOC, D], F32, tag="alpha_nat")
        nc.sync.dma_start(alpha_nat[:], alpha_dram[:, 0, :])
        amax = pp.tile([BLOC, 1], F32, tag="amax")
        nc.vector.tensor_reduce(
            out=amax[:], in_=alpha_nat[:], axis=mybir.AxisListType.X, op=AL.max,
        )
        amask = pp.tile([BLOC, D], F32, tag="amask")
        nc.vector.tensor_scalar(
            out=amask[:], in0=alpha_nat[:], scalar1=amax[:, 0:1], scalar2=None,
            op0=AL.is_ge,
        )
        af = pp.tile([BLOC, D], F32, tag="af")
        nc.vector.scalar_tensor_tensor(
            out=af[:], in0=amask[:], scalar=-BIG, in1=iota_big_nat[:],
            op0=AL.mult, op1=AL.add,
        )
        last_tag = pp.tile([BLOC, 1], F32, tag="last_tag")
        nc.vector.tensor_reduce(
            out=last_tag[:], in_=af[:], axis=mybir.AxisListType.X, op=AL.min,
        )

        # ---- backward-pass masks ----
        lm1 = pp.tile([BLOC, 1], F32, tag="lm1")
        nc.vector.tensor_scalar(
            out=lm1[:], in0=l_nat[:], scalar1=-1.0, scalar2=None, op0=AL.add,
        )
        inj = pp.tile([BLOC, T], F32, tag="inj")        # t == L-1
        nc.vector.tensor_scalar(
            out=inj[:], in0=iota_t_nat[:], scalar1=lm1[:, 0:1], scalar2=None,
            op0=AL.is_equal,
        )
        omj = pp.tile([BLOC, T], F32, tag="omj")        # 1 - inj
        nc.vector.tensor_scalar(
            out=omj[:], in0=inj[:], scalar1=-1.0, scalar2=1.0, op0=AL.mult, op1=AL.add,
        )
        lt_inj = pp.tile([BLOC, T], F32, tag="lt_inj")  # inj * last_tag
        nc.vector.tensor_scalar(
            out=lt_inj[:], in0=inj[:], scalar1=last_tag[:, 0:1], scalar2=None,
            op0=AL.mult,
        )
        ltinj2 = pp.tile([BLOC, T], F32, tag="ltinj2")   # BIG*omj + lt_inj
        nc.vector.scalar_tensor_tensor(
            out=ltinj2[:], in0=omj[:], scalar=BIG, in1=lt_inj[:],
            op0=AL.mult, op1=AL.add,
        )
        valid_nat = pp.tile([BLOC, T], F32, tag="valid_nat")
        nc.vector.tensor_scalar(
            out=valid_nat[:], in0=iota_t_nat[:], scalar1=l_nat[:, 0:1], scalar2=None,
            op0=AL.is_lt,
        )

        # ---- backward chain ----
        tagsq = pp.tile([BLOC, T], F32, tag="tagsq")
        h = pp.tile([BLOC, D], F32, tag="h")
        m = pp.tile([BLOC, 1], F32, tag="m")
        junk = pp.tile([BLOC, D], F32, tag="junk")

        # t = T-1 init: q = 0*omj + lt_inj ; h = (iota48 == q)
        nc.vector.memset(m[:], 0.0)
        nc.vector.scalar_tensor_tensor(
            out=tagsq[:, T - 1:T], in0=m[:], scalar=omj[:, T - 1:T],
            in1=ltinj2[:, T - 1:T], op0=AL.mult, op1=AL.add,
        )
        nc.vector.tensor_scalar(
            out=h[:], in0=iota48_nat[:], scalar1=tagsq[:, T - 1:T], scalar2=None,
            op0=AL.is_equal,
        )

        t_hi = T - 1  # highest bp index used is T-1
        for c0 in range(t_hi, 0, -CKB):
            ckb = min(CKB, c0)     # bp indices c0, c0-1, ..., c0-ckb+1 (>=1)
            bpb = cp.tile([BLOC, CKB, D], BF16, tag="bpb")
            bp_v = bp_dram_il.rearrange("(b h) t c -> b h t c", h=2)
            for hh in range(2):
                nc.sync.dma_start(
                    bpb[:, 0:ckb, HALF * hh:HALF * (hh + 1)],
                    bp_v[:, hh, c0 - ckb + 1:c0 + 1, :],
                )
            for tp1 in range(c0, c0 - ckb, -1):
                t = tp1 - 1
                kk = tp1 - (c0 - ckb + 1)
                nc.vector.scalar_tensor_tensor(
                    out=junk[:], in0=bpb[:, kk, :], scalar=1.0, in1=h[:],
                    op0=AL.mult, op1=AL.mult, accum_out=m[:],
                )
                nc.vector.scalar_tensor_tensor(
                    out=tagsq[:, t:t + 1], in0=m[:], scalar=omj[:, t:t + 1],
                    in1=ltinj2[:, t:t + 1], op0=AL.mult, op1=AL.add,
                )
                if t > 0:
                    nc.vector.tensor_scalar(
                        out=h[:], in0=iota48_nat[:], scalar1=tagsq[:, t:t + 1],
                        scalar2=None, op0=AL.is_equal,
                    )

        # ---- final masking + cast + store ----
        tags_f = pp.tile([BLOC, T], F32, tag="tags_f")
        nc.vector.tensor_mul(out=tags_f[:], in0=tagsq[:], in1=valid_nat[:])
        tags_i = pp.tile([BLOC, T], I32, tag="tags_i")
        nc.vector.tensor_copy(out=tags_i[:], in_=tags_f[:])
        nc.sync.dma_start(tags_out, tags_i[:])


# ---------------------------------------------------------------------------
# self-contained harness: build once, shard, run SPMD on 8 cores, unshard
# ---------------------------------------------------------------------------
import concourse.bacc as bacc
from concourse.bass_utils import run_bass_kernel_spmd

B = 512
T = 512
N_CORES = 8


def _input_specs():
    return {
        "logits_il": ([128, T, HALF], F32),
        "l_il": ([128, 1], F32),
        "l_nat": ([BLOC, 1], F32),
        "trans_rep": ([128, HALF * D], F32),
        "iota_neg": ([128, HALF * D], BF16),
        "iota_t_il": ([128, T], F32),
        "iota_t_nat": ([BLOC, T], F32),
        "iota48_nat": ([BLOC, D], F32),
        "iota_big_nat": ([BLOC, D], F32),
    }


_NC = None


def _build_nc():
    global _NC
    if _NC is not None:
        return _NC
    nc = bacc.Bacc(
        "TRN2",
        target_bir_lowering=False,
        debug=False,
        enable_asserts=True,
        num_devices=N_CORES,
    )
    ins = {
        name: nc.dram_tensor(name, shape, dt, kind="ExternalInput").ap()
        for name, (shape, dt) in _input_specs().items()
    }
    outs = {"tags": nc.dram_tensor("tags", [BLOC, T], I32, kind="ExternalOutput").ap()}
    with TileContext(nc) as tc:
        crf_kernel(tc, outs, ins, T=T)
    nc.compile()
    _NC = nc
    return nc


def kernel(logits, sent_lengths, crf_params):
    logits = np.asarray(logits, dtype=np.float32)
    sent_lengths = np.asarray(sent_lengths)
    consts = make_consts(crf_params, T)

    nc = _build_nc()
    in_maps = []
    for core in range(N_CORES):
        lg = logits[core * BLOC:(core + 1) * BLOC]
        sl = sent_lengths[core * BLOC:(core + 1) * BLOC]
        in_maps.append(make_core_inputs(lg, sl, consts))

    br = run_bass_kernel_spmd(nc, in_maps, core_ids=list(range(N_CORES)))
    out = np.concatenate(
        [br.results[core]["tags"] for core in range(N_CORES)], axis=0
    )
    return out.astype(np.int32)



# revision 2
# speedup vs baseline: 1.0131x; 1.0131x over previous
"""nn_CRFLayer: CRF Viterbi decode on 8 Trainium2 NeuronCores — packed version.

Key idea: sent_lengths are uniform 1..512 (mean ~256), so half of all (b,t)
positions are padding. The host bin-packs the 512 sequences into 256 "bins"
(lane-slots) of T_PACK steps (multiple sequences concatenated in time, with
alpha resets at the boundaries); each core runs 32 bins. Partition layout:
p = 4*bin_local + q, q = c-quarter (4 copies of each bin), so each lane does
12 cur x 48 prev per step instead of 24 x 48 — half the element work of the
unpacked kernel, with no padded steps.

All heavy ops run on the DVE (the Pool engine's software TTs contend with
DVE for SBUF and slow both ~2x when overlapped; measured). Per step:
  add (scores = trans_q + alpha_bcast), segmented reduce_max, one fused
  scalar_tensor_tensor alpha update (maxv * not_reset + emit — handles both
  the recurrence and the boundary reset), two stream_shuffles (4-way
  alpha all-gather). Backpointer extraction (is_ge mask -> bf16 mult by
  encoded iota -> segmented reduce_min, exact first-index argmax) is
  batched x4 steps and lags the alpha chain.

Sequence boundaries in the backward chase are handled with constant-map
backpointers: a batched post-pass computes AT[bin, t] = argmax_c alpha[t]
for every t, and the backward merges AT[t-1] into the bp stream at boundary
steps, so the chase "snaps" to each sequence's last tag with a plain 2-op
per-step chain (one-hot dot-accumulate + is_equal regeneration).

All value-producing float ops are the same single fp32 adds as the
reference, so decoded tags match bitwise-exactly.
"""

import numpy as np
import ml_dtypes

import concourse.bass as bass
import concourse.mybir as mybir
from concourse.tile import TileContext

AL = mybir.AluOpType
F32 = mybir.dt.float32
BF16 = mybir.dt.bfloat16
I32 = mybir.dt.int32

D = 48
Q = 12              # cur per lane
NBIN = 32           # bins per core
N_CORES = 8
B = 512
T_IN = 512
CK = 16             # forward DMA chunk (steps)
BATCH = 8           # bp-extraction batch (steps)
CKB = 32            # backward chunk (steps)

ROT1_MASK = [(i & ~3) | ((i + 1) & 3) for i in range(32)]
ROT2_MASK = [(i & ~3) | ((i + 2) & 3) for i in range(32)]


# ---------------------------------------------------------------------------
# host-side packing
# ---------------------------------------------------------------------------

def pack_bins(lengths: np.ndarray) -> tuple[int, list[list[tuple[int, int, int]]]]:
    """FFD-pack sequences into NBIN*N_CORES bins. Returns (T_PACK, bins) where
    bins[i] = [(orig_idx, start, length), ...]."""
    nbins = NBIN * N_CORES
    order = np.argsort(-lengths, kind="stable")
    t_pack = max(512, int(np.ceil(lengths.sum() / nbins / 16) * 16))
    while True:
        fills = [0] * nbins
        bins = [[] for _ in range(nbins)]
        ok = True
        for idx in order:
            L = int(lengths[idx])
            for i in range(nbins):
                if fills[i] + L <= t_pack:
                    bins[i].append((int(idx), fills[i], L))
                    fills[i] += L
                    break
            else:
                ok = False
                break
        if ok:
            return t_pack, bins
        t_pack += 16


def make_consts(trans: np.ndarray, t_pack: int) -> dict[str, np.ndarray]:
    trans = np.asarray(trans, dtype=np.float32)
    trans_q = np.zeros((128, Q, D), dtype=np.float32)
    iota_q = np.zeros((128, BATCH, Q, D), dtype=np.float32)
    for q in range(4):
        prev = (Q * q + np.arange(D)) % D          # jslot -> real j
        cur = Q * q + np.arange(Q)
        block = trans[prev][:, cur].T              # [Q, D]
        enc = np.broadcast_to((prev - 64.0)[None, None, :], (BATCH, Q, D))
        for bl in range(NBIN):
            p = 4 * bl + q
            trans_q[p] = block
            iota_q[p] = enc
    at_enc = np.broadcast_to(
        (np.arange(D, dtype=np.float32) - 64.0)[None, :], (128, D)
    ).copy()                                        # (q,c) -> 12q+c - 64
    iota_bw = np.broadcast_to(
        (np.arange(D, dtype=np.float32) - 64.0)[None, :], (NBIN, D)
    ).copy()
    return {
        "trans_q": trans_q.reshape(128, Q * D),
        "iota_q": iota_q.reshape(128, BATCH * Q * D).astype(ml_dtypes.bfloat16),
        "at_enc": at_enc,
        "iota_bw": iota_bw,
    }


def make_core_inputs(logits, bins_core, consts, t_pack) -> dict[str, np.ndarray]:
    """bins_core: 32 bins for this core, each [(orig_idx, start, L), ...]."""
    emit = np.zeros((128, t_pack, Q), dtype=np.float32)
    nr = np.ones((NBIN, t_pack), dtype=np.float32)
    bm = np.zeros((NBIN, t_pack), dtype=np.float32)
    for bl, seqs in enumerate(bins_core):
        fill = 0
        for (idx, s, L) in seqs:
            for q in range(4):
                emit[4 * bl + q, s:s + L, :] = logits[idx, :L, Q * q:Q * q + Q]
            nr[bl, s] = 0.0
            if s >= 1:
                bm[bl, s] = 1.0
            fill = s + L
        if fill < t_pack:          # junk start is a boundary too
            nr[bl, fill] = 0.0
            if fill >= 1:
                bm[bl, fill] = 1.0
    nr_il = np.repeat(nr, 4, axis=0)               # [128, T]
    nbm = 1.0 - bm
    return dict(
        consts,
        emit=np.ascontiguousarray(emit),
        nr_il=np.ascontiguousarray(nr_il),
        bm=np.ascontiguousarray(bm),
        nbm=np.ascontiguousarray(nbm),
    )


# ---------------------------------------------------------------------------
# kernel
# ---------------------------------------------------------------------------

def crf_kernel(tc: TileContext, outs, ins, T: int):
    nc = tc.nc
    TC = T // 4                     # per-(bl,tc) chunk for the AT pass

    emit_d = ins["emit"]            # [128, T, Q] dram f32
    tags_out = outs["tags"]         # [NBIN, T] dram i32

    bp_dram = nc.dram_tensor("bp_scratch", [128, T, Q], BF16, kind="Internal").ap()
    ah_dram = nc.dram_tensor("ah_scratch", [128, T, Q], F32, kind="Internal").ap()
    at_dram = nc.dram_tensor("at_scratch", [NBIN, T + 1], F32, kind="Internal").ap()

    with (
        tc.tile_pool(name="persist", bufs=1) as pp,
        tc.tile_pool(name="chunks", bufs=3) as cp,
        tc.tile_pool(name="work", bufs=1) as wp,
    ):
        # ---- persistent constants ----
        trans_q = pp.tile([128, Q, D], F32, tag="trans_q")
        nc.sync.dma_start(trans_q[:].rearrange("p a b -> p (a b)"), ins["trans_q"])
        iota_q = pp.tile([128, BATCH, Q, D], BF16, tag="iota_q")
        nc.sync.dma_start(iota_q[:].rearrange("p a b c -> p (a b c)"), ins["iota_q"])
        at_enc = pp.tile([128, D], F32, tag="at_enc")
        nc.sync.dma_start(at_enc[:], ins["at_enc"])
        iota_bw = pp.tile([NBIN, D], F32, tag="iota_bw")
        nc.sync.dma_start(iota_bw[:], ins["iota_bw"])
        nr_il = pp.tile([128, T], F32, tag="nr_il")
        nc.sync.dma_start(nr_il[:], ins["nr_il"])
        bm = pp.tile([NBIN, T], F32, tag="bm")
        nc.sync.dma_start(bm[:], ins["bm"])
        nbm = pp.tile([NBIN, T], F32, tag="nbm")
        nc.sync.dma_start(nbm[:], ins["nbm"])

        aring = pp.tile([128, 8, D], F32, tag="aring")
        nc.vector.memset(aring[:], 0.0)

        srings = [pp.tile([128, BATCH, Q, D], F32, name=f"sring{i}", tag=f"sring{i}") for i in range(2)]
        mrings = [pp.tile([128, BATCH, Q], F32, name=f"mring{i}", tag=f"mring{i}") for i in range(2)]

        # ---- forward scan ----
        emit_tiles = {}

        def fetch_emit(t0):
            et = cp.tile([128, CK, Q], F32, name=f"emit_{t0}", tag="emit_ch")
            nc.sync.dma_start(et[:], emit_d[:, t0:t0 + CK, :])
            emit_tiles[t0] = et

        fetch_emit(0)
        for t0 in range(0, T, CK):
            if t0 + CK < T:
                fetch_emit(t0 + CK)
            emit_ch = emit_tiles.pop(t0)
            bp_ch = cp.tile([128, CK, Q], BF16, tag="bp_ch")
            for t in range(t0, t0 + CK):
                r = (t // BATCH) % 2
                k = t % BATCH
                kc = t - t0
                a_prev = aring[:, (t + 7) % 8, :]
                a_next = aring[:, t % 8, :]
                sc = srings[r]
                mv = mrings[r]
                a_b = a_prev.unsqueeze(1).broadcast_to([128, Q, D])
                nc.vector.tensor_tensor(out=sc[:, k], in0=trans_q[:], in1=a_b, op=AL.add)
                nc.vector.tensor_reduce(
                    out=mv[:, k], in_=sc[:, k], axis=mybir.AxisListType.X, op=AL.max,
                )
                # alpha = maxv * not_reset + emit  (reset -> alpha = emit)
                nc.vector.scalar_tensor_tensor(
                    out=a_next[0:128, 0:Q], in0=mv[:, k], scalar=nr_il[:, t:t + 1],
                    in1=emit_ch[:, kc, :], op0=AL.mult, op1=AL.add,
                )
                nc.vector.stream_shuffle(a_next[0:128, Q:2 * Q], a_next[0:128, 0:Q], mask=ROT1_MASK)
                nc.vector.stream_shuffle(a_next[0:128, 2 * Q:4 * Q], a_next[0:128, 0:2 * Q], mask=ROT2_MASK)
                if t % 4 == 3:
                    s4 = (t - 3) % 8
                    nc.sync.dma_start(
                        ah_dram[:, t - 3:t + 1, :], aring[:, s4:s4 + 4, 0:Q]
                    )
                if k == BATCH - 1:
                    # batched bp extraction for steps t-BATCH+1..t
                    mask4 = wp.tile([128, BATCH, Q, D], BF16, tag="mask4")
                    f4 = wp.tile([128, BATCH, Q, D], BF16, tag="f4")
                    sc_v = sc[:].rearrange("p b c j -> p (b c) j")
                    mv_v = mv[:].rearrange("p b c -> p (b c)").unsqueeze(2)
                    nc.vector.tensor_tensor(
                        out=mask4[:].rearrange("p b c j -> p (b c) j"), in0=sc_v,
                        in1=mv_v.broadcast_to([128, BATCH * Q, D]), op=AL.is_ge,
                    )
                    nc.vector.tensor_tensor(out=f4[:], in0=mask4[:], in1=iota_q[:], op=AL.mult)
                    # segmented min via bf16 TT tree (2x DVE mode; TR has no 2x)
                    fv = f4[:].rearrange("p b c j -> p (b c) j")
                    nc.vector.tensor_tensor(
                        out=fv[:, :, 0:24], in0=fv[:, :, 0:24], in1=fv[:, :, 24:48], op=AL.min)
                    nc.vector.tensor_tensor(
                        out=fv[:, :, 0:12], in0=fv[:, :, 0:12], in1=fv[:, :, 12:24], op=AL.min)
                    nc.vector.tensor_tensor(
                        out=fv[:, :, 0:6], in0=fv[:, :, 0:6], in1=fv[:, :, 6:12], op=AL.min)
                    nc.vector.tensor_tensor(
                        out=fv[:, :, 0:3], in0=fv[:, :, 0:3], in1=fv[:, :, 3:6], op=AL.min)
                    nc.vector.tensor_tensor(
                        out=fv[:, :, 0:1], in0=fv[:, :, 0:1], in1=fv[:, :, 1:2], op=AL.min)
                    bp_out = bp_ch[:, kc - BATCH + 1:kc + 1, :].rearrange(
                        "p t c -> p (t c)").unsqueeze(2)
                    nc.vector.tensor_tensor(
                        out=bp_out, in0=fv[:, :, 0:1], in1=fv[:, :, 2:3], op=AL.min)
            nc.sync.dma_start(bp_dram[:, t0:t0 + CK, :], bp_ch[:])

        # ---- AT pass: AT[bin, t] = enc(first-argmax_c alpha[bin, t, :]) ----
        # atile partitions = (tc, b): lane tc*32+b covers t in [tc*TC, (tc+1)*TC)
        ah_v4 = ah_dram.rearrange("(b q) t c -> b q t c", q=4)
        with tc.tile_pool(name="atpool", bufs=1) as ap:
            atile = ap.tile([128, TC, 4, Q], F32, tag="atile")
            for tc4 in range(4):
                for q in range(4):
                    nc.sync.dma_start(
                        atile[tc4 * NBIN:(tc4 + 1) * NBIN, :, q, :],
                        ah_v4[:, q, tc4 * TC:(tc4 + 1) * TC, :],
                    )
            atmax = ap.tile([128, TC], F32, tag="atmax")
            nc.vector.tensor_reduce(
                out=atmax[:], in_=atile[:], axis=mybir.AxisListType.XY, op=AL.max,
            )
            atmask = ap.tile([128, TC, D], BF16, tag="atmask")
            nc.vector.tensor_tensor(
                out=atmask[:], in0=atile[:].rearrange("p t q c -> p t (q c)"),
                in1=atmax[:].unsqueeze(2).broadcast_to([128, TC, D]), op=AL.is_ge,
            )
            atf = ap.tile([128, TC, D], BF16, tag="atf")
            nc.vector.tensor_tensor(
                out=atf[:], in0=atmask[:],
                in1=at_enc[:].unsqueeze(1).broadcast_to([128, TC, D]),
                op=AL.mult,
            )
            at_all = ap.tile([128, TC], F32, tag="at_all")
            nc.vector.tensor_reduce(
                out=at_all[:], in_=atf[:], axis=mybir.AxisListType.X, op=AL.min,
            )
            # at_dram[bin, 1 + t] = AT[bin, t]
            for tc4 in range(4):
                nc.sync.dma_start(
                    at_dram[:, 1 + tc4 * TC:1 + (tc4 + 1) * TC],
                    at_all[tc4 * NBIN:(tc4 + 1) * NBIN, :],
                )

        # ---- backward chase ----
        h = pp.tile([NBIN, D], F32, tag="h")
        tagsq = pp.tile([NBIN, T], F32, tag="tagsq")
        junk = pp.tile([NBIN, D], F32, tag="junk")
        at_last = pp.tile([NBIN, 1], F32, tag="at_last")
        nc.sync.dma_start(at_last[:], at_dram[:, T:T + 1])
        nc.vector.tensor_copy(out=tagsq[:, T - 1:T], in_=at_last[:])
        nc.vector.tensor_scalar(
            out=h[:], in0=iota_bw[:], scalar1=at_last[:, 0:1], scalar2=None,
            op0=AL.is_equal,
        )

        bp_v = bp_dram.rearrange("(b q) t c -> b q t c", q=4)
        bwp_ctx = tc.tile_pool(name="bwp", bufs=2)
        bwp = bwp_ctx.__enter__()
        for c0 in range(T - 1, 0, -CKB):
            ckb = min(CKB, c0)      # bp indices c0, c0-1, ..., c0-ckb+1 (>=1)
            lo = c0 - ckb + 1
            bpb = bwp.tile([NBIN, CKB, D], BF16, tag="bpb")
            for q in range(4):
                nc.sync.dma_start(
                    bpb[:, 0:ckb, Q * q:Q * (q + 1)], bp_v[:, q, lo:c0 + 1, :]
                )
            atp = bwp.tile([NBIN, CKB], F32, tag="atp")
            nc.sync.dma_start(atp[:, 0:ckb], at_dram[:, lo:c0 + 1])
            # merge: bpb2 = bpb * nbm + AT[t-1] * bm   (boundary const-maps)
            bpb2 = bwp.tile([NBIN, CKB, D], F32, tag="bpb2")
            nc.vector.tensor_tensor(
                out=bpb2[:, 0:ckb, :], in0=bpb[:, 0:ckb, :],
                in1=nbm[:, lo:c0 + 1].unsqueeze(2).broadcast_to([NBIN, ckb, D]),
                op=AL.mult,
            )
            atpm = bwp.tile([NBIN, CKB], F32, tag="atpm")
            nc.vector.tensor_tensor(
                out=atpm[:, 0:ckb], in0=atp[:, 0:ckb],
                in1=bm[:, lo:c0 + 1], op=AL.mult,
            )
            nc.vector.tensor_tensor(
                out=bpb2[:, 0:ckb, :], in0=bpb2[:, 0:ckb, :],
                in1=atpm[:, 0:ckb].unsqueeze(2).broadcast_to([NBIN, ckb, D]), op=AL.add,
            )
            for t in range(c0, lo - 1, -1):
                kk = t - lo
                nc.vector.scalar_tensor_tensor(
                    out=junk[:], in0=bpb2[:, kk, :], scalar=1.0, in1=h[:],
                    op0=AL.mult, op1=AL.mult, accum_out=tagsq[:, t - 1:t],
                )
                if t > 1:
                    nc.vector.tensor_scalar(
                        out=h[:], in0=iota_bw[:], scalar1=tagsq[:, t - 1:t],
                        scalar2=None, op0=AL.is_equal,
                    )

        bwp_ctx.__exit__(None, None, None)

        # ---- decode (+64) + cast + store ----
        tags_f = pp.tile([NBIN, T], F32, tag="tags_f")
        nc.vector.tensor_scalar(
            out=tags_f[:], in0=tagsq[:], scalar1=64.0, scalar2=None, op0=AL.add,
        )
        tags_i = pp.tile([NBIN, T], I32, tag="tags_i")
        nc.vector.tensor_copy(out=tags_i[:], in_=tags_f[:])
        nc.sync.dma_start(tags_out, tags_i[:])


# ---------------------------------------------------------------------------
# self-contained harness
# ---------------------------------------------------------------------------
import concourse.bacc as bacc
from concourse.bass_utils import run_bass_kernel_spmd

_NC_CACHE: dict[int, object] = {}


def _input_specs(t_pack):
    return {
        "emit": ([128, t_pack, Q], F32),
        "nr_il": ([128, t_pack], F32),
        "bm": ([NBIN, t_pack], F32),
        "nbm": ([NBIN, t_pack], F32),
        "trans_q": ([128, Q * D], F32),
        "iota_q": ([128, BATCH * Q * D], BF16),
        "at_enc": ([128, D], F32),
        "iota_bw": ([NBIN, D], F32),
    }


def _build_nc(t_pack):
    if t_pack in _NC_CACHE:
        return _NC_CACHE[t_pack]
    nc = bacc.Bacc(
        "TRN2",
        target_bir_lowering=False,
        debug=False,
        enable_asserts=True,
        num_devices=N_CORES,
    )
    ins = {
        name: nc.dram_tensor(name, shape, dt, kind="ExternalInput").ap()
        for name, (shape, dt) in _input_specs(t_pack).items()
    }
    outs = {"tags": nc.dram_tensor("tags", [NBIN, t_pack], I32, kind="ExternalOutput").ap()}
    with TileContext(nc) as tc:
        crf_kernel(tc, outs, ins, T=t_pack)
    nc.compile()
    _NC_CACHE[t_pack] = nc
    return nc


def _prepare(logits, sent_lengths, crf_params):
    logits = np.asarray(logits, dtype=np.float32)
    lengths = np.asarray(sent_lengths).astype(np.int64)
    t_pack, bins = pack_bins(lengths)
    consts = make_consts(crf_params, t_pack)
    in_maps = []
    for core in range(N_CORES):
        bins_core = bins[core * NBIN:(core + 1) * NBIN]
        in_maps.append(make_core_inputs(logits, bins_core, consts, t_pack))
    return t_pack, bins, in_maps


def _unpack(results, bins, lengths, t_pack):
    out = np.zeros((B, T_IN), dtype=np.int32)
    for core in range(N_CORES):
        tags = results[core]["tags"]            # [NBIN, t_pack] i32
        for bl, seqs in enumerate(bins[core * NBIN:(core + 1) * NBIN]):
            for (idx, s, L) in seqs:
                out[idx, 0:L] = tags[bl, s:s + L]
    return out


def kernel(logits, sent_lengths, crf_params):
    lengths = np.asarray(sent_lengths).astype(np.int64)
    t_pack, bins, in_maps = _prepare(logits, sent_lengths, crf_params)
    nc = _build_nc(t_pack)
    br = run_bass_kernel_spmd(nc, in_maps, core_ids=list(range(N_CORES)))
    return _unpack(br.results, bins, lengths, t_pack)


# revision 3
# speedup vs baseline: 1.0237x; 1.0105x over previous
"""nn_CRFLayer: CRF Viterbi decode on 8 Trainium2 NeuronCores — packed version.

Key idea: sent_lengths are uniform 1..512 (mean ~256), so half of all (b,t)
positions are padding. The host bin-packs the 512 sequences into 256 "bins"
(lane-slots) of T_PACK steps (multiple sequences concatenated in time, with
alpha resets at the boundaries); each core runs 32 bins. Partition layout:
p = 4*bin_local + q, q = c-quarter (4 copies of each bin), so each lane does
12 cur x 48 prev per step instead of 24 x 48 — half the element work of the
unpacked kernel, with no padded steps.

All heavy ops run on the DVE (the Pool engine's software TTs contend with
DVE for SBUF and slow both ~2x when overlapped; measured). Per step:
  add (scores = trans_q + alpha_bcast), segmented reduce_max, one fused
  scalar_tensor_tensor alpha update (maxv * not_reset + emit — handles both
  the recurrence and the boundary reset), two stream_shuffles (4-way
  alpha all-gather). Backpointer extraction (is_ge mask -> bf16 mult by
  encoded iota -> segmented reduce_min, exact first-index argmax) is
  batched x4 steps and lags the alpha chain.

Sequence boundaries in the backward chase are handled with constant-map
backpointers: a batched post-pass computes AT[bin, t] = argmax_c alpha[t]
for every t, and the backward merges AT[t-1] into the bp stream at boundary
steps, so the chase "snaps" to each sequence's last tag with a plain 2-op
per-step chain (one-hot dot-accumulate + is_equal regeneration).

All value-producing float ops are the same single fp32 adds as the
reference, so decoded tags match bitwise-exactly.
"""

import numpy as np
import ml_dtypes

import concourse.bass as bass
import concourse.mybir as mybir
from concourse.tile import TileContext

AL = mybir.AluOpType
F32 = mybir.dt.float32
BF16 = mybir.dt.bfloat16
I32 = mybir.dt.int32

D = 48
Q = 12              # cur per lane
NBIN = 32           # bins per core
N_CORES = 8
B = 512
T_IN = 512
CK = 16             # forward DMA chunk (steps)
BATCH = 8           # bp-extraction batch (steps)
CKB = 32            # backward chunk (steps)

ROT1_MASK = [(i & ~3) | ((i + 1) & 3) for i in range(32)]
ROT2_MASK = [(i & ~3) | ((i + 2) & 3) for i in range(32)]


# ---------------------------------------------------------------------------
# host-side packing
# ---------------------------------------------------------------------------

def pack_bins(lengths: np.ndarray) -> tuple[int, list[list[tuple[int, int, int]]]]:
    """FFD-pack sequences into NBIN*N_CORES bins. Returns (T_PACK, bins) where
    bins[i] = [(orig_idx, start, length), ...]."""
    nbins = NBIN * N_CORES
    order = np.argsort(-lengths, kind="stable")
    t_pack = max(512, int(np.ceil(lengths.sum() / nbins / 16) * 16))
    while True:
        fills = [0] * nbins
        bins = [[] for _ in range(nbins)]
        ok = True
        for idx in order:
            L = int(lengths[idx])
            for i in range(nbins):
                if fills[i] + L <= t_pack:
                    bins[i].append((int(idx), fills[i], L))
                    fills[i] += L
                    break
            else:
                ok = False
                break
        if ok:
            return t_pack, bins
        t_pack += 16


def make_consts(trans: np.ndarray, t_pack: int) -> dict[str, np.ndarray]:
    trans = np.asarray(trans, dtype=np.float32)
    trans_q = np.zeros((128, Q, D), dtype=np.float32)
    iota_q = np.zeros((128, BATCH, Q, D), dtype=np.float32)
    for q in range(4):
        prev = (Q * q + np.arange(D)) % D          # jslot -> real j
        cur = Q * q + np.arange(Q)
        block = trans[prev][:, cur].T              # [Q, D]
        enc = np.broadcast_to((prev - 64.0)[None, None, :], (BATCH, Q, D))
        for bl in range(NBIN):
            p = 4 * bl + q
            trans_q[p] = block
            iota_q[p] = enc
    at_enc = np.broadcast_to(
        (np.arange(D, dtype=np.float32) - 64.0)[None, :], (128, D)
    ).copy()                                        # (q,c) -> 12q+c - 64
    iota_bw = np.broadcast_to(
        (np.arange(D, dtype=np.float32) - 64.0)[None, :], (NBIN, D)
    ).copy()
    return {
        "trans_q": trans_q.reshape(128, Q * D),
        "iota_q": iota_q.reshape(128, BATCH * Q * D).astype(ml_dtypes.bfloat16),
        "at_enc": at_enc,
        "iota_bw": iota_bw,
    }


def make_core_inputs(logits, bins_core, consts, t_pack) -> dict[str, np.ndarray]:
    """bins_core: 32 bins for this core, each [(orig_idx, start, L), ...]."""
    emit = np.zeros((128, t_pack, Q), dtype=np.float32)
    nr = np.ones((NBIN, t_pack), dtype=np.float32)
    bm = np.zeros((NBIN, t_pack), dtype=np.float32)
    for bl, seqs in enumerate(bins_core):
        fill = 0
        for (idx, s, L) in seqs:
            for q in range(4):
                emit[4 * bl + q, s:s + L, :] = logits[idx, :L, Q * q:Q * q + Q]
            nr[bl, s] = 0.0
            if s >= 1:
                bm[bl, s] = 1.0
            fill = s + L
        if fill < t_pack:          # junk start is a boundary too
            nr[bl, fill] = 0.0
            if fill >= 1:
                bm[bl, fill] = 1.0
    nr_il = np.repeat(nr, 4, axis=0)               # [128, T]
    nbm = 1.0 - bm
    return dict(
        consts,
        emit=np.ascontiguousarray(emit),
        nr_il=np.ascontiguousarray(nr_il),
        bm=np.ascontiguousarray(bm),
        nbm=np.ascontiguousarray(nbm),
    )


# ---------------------------------------------------------------------------
# kernel
# ---------------------------------------------------------------------------

def crf_kernel(tc: TileContext, outs, ins, T: int):
    nc = tc.nc
    TC = T // 4                     # per-(bl,tc) chunk for the AT pass

    emit_d = ins["emit"]            # [128, T, Q] dram f32
    tags_out = outs["tags"]         # [NBIN, T] dram i32

    bp_dram = nc.dram_tensor("bp_scratch", [128, T, Q], BF16, kind="Internal").ap()
    ah_dram = nc.dram_tensor("ah_scratch", [128, T, Q], F32, kind="Internal").ap()
    at_dram = nc.dram_tensor("at_scratch", [NBIN, T + 1], F32, kind="Internal").ap()

    with (
        tc.tile_pool(name="persist", bufs=1) as pp,
        tc.tile_pool(name="chunks", bufs=3) as cp,
        tc.tile_pool(name="work", bufs=1) as wp,
    ):
        # ---- persistent constants ----
        trans_q = pp.tile([128, Q, D], F32, tag="trans_q")
        nc.sync.dma_start(trans_q[:].rearrange("p a b -> p (a b)"), ins["trans_q"])
        iota_q = pp.tile([128, BATCH, Q, D], BF16, tag="iota_q")
        nc.sync.dma_start(iota_q[:].rearrange("p a b c -> p (a b c)"), ins["iota_q"])
        at_enc = pp.tile([128, D], F32, tag="at_enc")
        nc.sync.dma_start(at_enc[:], ins["at_enc"])
        iota_bw = pp.tile([NBIN, D], F32, tag="iota_bw")
        nc.sync.dma_start(iota_bw[:], ins["iota_bw"])
        nr_il = pp.tile([128, T], F32, tag="nr_il")
        nc.sync.dma_start(nr_il[:], ins["nr_il"])
        bm = pp.tile([NBIN, T], F32, tag="bm")
        nc.sync.dma_start(bm[:], ins["bm"])
        nbm = pp.tile([NBIN, T], F32, tag="nbm")
        nc.sync.dma_start(nbm[:], ins["nbm"])

        aring = pp.tile([128, 8, D], F32, tag="aring")
        nc.vector.memset(aring[:], 0.0)

        srings = [pp.tile([128, BATCH, Q, D], F32, name=f"sring{i}", tag=f"sring{i}") for i in range(2)]
        mrings = [pp.tile([128, BATCH, Q], F32, name=f"mring{i}", tag=f"mring{i}") for i in range(2)]

        # ---- forward scan ----
        emit_tiles = {}

        def fetch_emit(t0):
            et = cp.tile([128, CK, Q], F32, name=f"emit_{t0}", tag="emit_ch")
            nc.sync.dma_start(et[:], emit_d[:, t0:t0 + CK, :])
            emit_tiles[t0] = et

        fetch_emit(0)
        for t0 in range(0, T, CK):
            if t0 + CK < T:
                fetch_emit(t0 + CK)
            emit_ch = emit_tiles.pop(t0)
            bp_ch = cp.tile([128, CK, Q], BF16, tag="bp_ch")
            for t in range(t0, t0 + CK):
                r = (t // BATCH) % 2
                k = t % BATCH
                kc = t - t0
                a_prev = aring[:, (t + 7) % 8, :]
                a_next = aring[:, t % 8, :]
                sc = srings[r]
                mv = mrings[r]
                a_b = a_prev.unsqueeze(1).broadcast_to([128, Q, D])
                nc.vector.tensor_tensor(out=sc[:, k], in0=trans_q[:], in1=a_b, op=AL.add)
                nc.vector.tensor_reduce(
                    out=mv[:, k], in_=sc[:, k], axis=mybir.AxisListType.X, op=AL.max,
                )
                # alpha = maxv * not_reset + emit  (reset -> alpha = emit)
                nc.vector.scalar_tensor_tensor(
                    out=a_next[0:128, 0:Q], in0=mv[:, k], scalar=nr_il[:, t:t + 1],
                    in1=emit_ch[:, kc, :], op0=AL.mult, op1=AL.add,
                )
                nc.vector.stream_shuffle(a_next[0:128, Q:2 * Q], a_next[0:128, 0:Q], mask=ROT1_MASK)
                nc.vector.stream_shuffle(a_next[0:128, 2 * Q:4 * Q], a_next[0:128, 0:2 * Q], mask=ROT2_MASK)
                if t % 4 == 3:
                    s4 = (t - 3) % 8
                    nc.sync.dma_start(
                        ah_dram[:, t - 3:t + 1, :], aring[:, s4:s4 + 4, 0:Q]
                    )
                if k == BATCH - 1:
                    # batched bp extraction for steps t-BATCH+1..t
                    mask4 = wp.tile([128, BATCH, Q, D], BF16, tag="mask4")
                    f4 = wp.tile([128, BATCH, Q, D], BF16, tag="f4")
                    sc_v = sc[:].rearrange("p b c j -> p (b c) j")
                    mv_v = mv[:].rearrange("p b c -> p (b c)").unsqueeze(2)
                    nc.vector.tensor_tensor(
                        out=mask4[:].rearrange("p b c j -> p (b c) j"), in0=sc_v,
                        in1=mv_v.broadcast_to([128, BATCH * Q, D]), op=AL.is_ge,
                    )
                    nc.vector.tensor_tensor(out=f4[:], in0=mask4[:], in1=iota_q[:], op=AL.mult)
                    # segmented min via bf16 TT tree (2x DVE mode; TR has no 2x)
                    fv = f4[:].rearrange("p b c j -> p (b c) j")
                    nc.vector.tensor_tensor(
                        out=fv[:, :, 0:24], in0=fv[:, :, 0:24], in1=fv[:, :, 24:48], op=AL.min)
                    nc.vector.tensor_tensor(
                        out=fv[:, :, 0:12], in0=fv[:, :, 0:12], in1=fv[:, :, 12:24], op=AL.min)
                    nc.vector.tensor_tensor(
                        out=fv[:, :, 0:6], in0=fv[:, :, 0:6], in1=fv[:, :, 6:12], op=AL.min)
                    nc.vector.tensor_tensor(
                        out=fv[:, :, 0:3], in0=fv[:, :, 0:3], in1=fv[:, :, 3:6], op=AL.min)
                    nc.vector.tensor_tensor(
                        out=fv[:, :, 0:1], in0=fv[:, :, 0:1], in1=fv[:, :, 1:2], op=AL.min)
                    bp_out = bp_ch[:, kc - BATCH + 1:kc + 1, :].rearrange(
                        "p t c -> p (t c)").unsqueeze(2)
                    nc.vector.tensor_tensor(
                        out=bp_out, in0=fv[:, :, 0:1], in1=fv[:, :, 2:3], op=AL.min)
            nc.sync.dma_start(bp_dram[:, t0:t0 + CK, :], bp_ch[:])

        # ---- AT pass: AT[bin, t] = enc(first-argmax_c alpha[bin, t, :]) ----
        # atile partitions = (tc, b): lane tc*32+b covers t in [tc*TC, (tc+1)*TC)
        ah_v4 = ah_dram.rearrange("(b q) t c -> b q t c", q=4)
        with tc.tile_pool(name="atpool", bufs=1) as ap:
            atile = ap.tile([128, TC, 4, Q], F32, tag="atile")
            for tc4 in range(4):
                for q in range(4):
                    nc.sync.dma_start(
                        atile[tc4 * NBIN:(tc4 + 1) * NBIN, :, q, :],
                        ah_v4[:, q, tc4 * TC:(tc4 + 1) * TC, :],
                    )
            atmax = ap.tile([128, TC], F32, tag="atmax")
            nc.vector.tensor_reduce(
                out=atmax[:], in_=atile[:], axis=mybir.AxisListType.XY, op=AL.max,
            )
            atmask = ap.tile([128, TC, D], BF16, tag="atmask")
            nc.vector.tensor_tensor(
                out=atmask[:], in0=atile[:].rearrange("p t q c -> p t (q c)"),
                in1=atmax[:].unsqueeze(2).broadcast_to([128, TC, D]), op=AL.is_ge,
            )
            atf = ap.tile([128, TC, D], BF16, tag="atf")
            nc.vector.tensor_tensor(
                out=atf[:], in0=atmask[:],
                in1=at_enc[:].unsqueeze(1).broadcast_to([128, TC, D]),
                op=AL.mult,
            )
            at_all = ap.tile([128, TC], F32, tag="at_all")
            nc.vector.tensor_reduce(
                out=at_all[:], in_=atf[:], axis=mybir.AxisListType.X, op=AL.min,
            )
            # at_dram[bin, 1 + t] = AT[bin, t]
            for tc4 in range(4):
                nc.sync.dma_start(
                    at_dram[:, 1 + tc4 * TC:1 + (tc4 + 1) * TC],
                    at_all[tc4 * NBIN:(tc4 + 1) * NBIN, :],
                )

        # ---- backward chase ----
        h = pp.tile([NBIN, D], F32, tag="h")
        tagsq = pp.tile([NBIN, T], F32, tag="tagsq")
        junk = pp.tile([NBIN, D], F32, tag="junk")
        at_last = pp.tile([NBIN, 1], F32, tag="at_last")
        nc.sync.dma_start(at_last[:], at_dram[:, T:T + 1])
        nc.vector.tensor_copy(out=tagsq[:, T - 1:T], in_=at_last[:])
        nc.vector.tensor_scalar(
            out=h[:], in0=iota_bw[:], scalar1=at_last[:, 0:1], scalar2=None,
            op0=AL.is_equal,
        )

        bp_v = bp_dram.rearrange("(b q) t c -> b q t c", q=4)
        bwp_ctx = tc.tile_pool(name="bwp", bufs=2)
        bwp = bwp_ctx.__enter__()
        for c0 in range(T - 1, 0, -CKB):
            ckb = min(CKB, c0)      # bp indices c0, c0-1, ..., c0-ckb+1 (>=1)
            lo = c0 - ckb + 1
            bpb = bwp.tile([NBIN, CKB, D], BF16, tag="bpb")
            for q in range(4):
                nc.sync.dma_start(
                    bpb[:, 0:ckb, Q * q:Q * (q + 1)], bp_v[:, q, lo:c0 + 1, :]
                )
            atp = bwp.tile([NBIN, CKB], F32, tag="atp")
            nc.sync.dma_start(atp[:, 0:ckb], at_dram[:, lo:c0 + 1])
            # merge: bpb2 = bpb * nbm + AT[t-1] * bm   (boundary const-maps)
            bpb2 = bwp.tile([NBIN, CKB, D], F32, tag="bpb2")
            nc.vector.tensor_tensor(
                out=bpb2[:, 0:ckb, :], in0=bpb[:, 0:ckb, :],
                in1=nbm[:, lo:c0 + 1].unsqueeze(2).broadcast_to([NBIN, ckb, D]),
                op=AL.mult,
            )
            atpm = bwp.tile([NBIN, CKB], F32, tag="atpm")
            nc.vector.tensor_tensor(
                out=atpm[:, 0:ckb], in0=atp[:, 0:ckb],
                in1=bm[:, lo:c0 + 1], op=AL.mult,
            )
            nc.vector.tensor_tensor(
                out=bpb2[:, 0:ckb, :], in0=bpb2[:, 0:ckb, :],
                in1=atpm[:, 0:ckb].unsqueeze(2).broadcast_to([NBIN, ckb, D]), op=AL.add,
            )
            for t in range(c0, lo - 1, -1):
                kk = t - lo
                nc.vector.scalar_tensor_tensor(
                    out=junk[:], in0=bpb2[:, kk, :], scalar=1.0, in1=h[:],
                    op0=AL.mult, op1=AL.mult, accum_out=tagsq[:, t - 1:t],
                )
                if t > 1:
                    nc.vector.tensor_tensor(
                        out=h[:], in0=iota_bw[:],
                        in1=tagsq[:, t - 1:t].broadcast_to([NBIN, D]),
                        op=AL.is_equal,
                    )

        bwp_ctx.__exit__(None, None, None)

        # ---- decode (+64) + cast + store ----
        tags_f = pp.tile([NBIN, T], F32, tag="tags_f")
        nc.vector.tensor_scalar(
            out=tags_f[:], in0=tagsq[:], scalar1=64.0, scalar2=None, op0=AL.add,
        )
        tags_i = pp.tile([NBIN, T], I32, tag="tags_i")
        nc.vector.tensor_copy(out=tags_i[:], in_=tags_f[:])
        nc.sync.dma_start(tags_out, tags_i[:])


# ---------------------------------------------------------------------------
# self-contained harness
# ---------------------------------------------------------------------------
import concourse.bacc as bacc
from concourse.bass_utils import run_bass_kernel_spmd

_NC_CACHE: dict[int, object] = {}


def _input_specs(t_pack):
    return {
        "emit": ([128, t_pack, Q], F32),
        "nr_il": ([128, t_pack], F32),
        "bm": ([NBIN, t_pack], F32),
        "nbm": ([NBIN, t_pack], F32),
        "trans_q": ([128, Q * D], F32),
        "iota_q": ([128, BATCH * Q * D], BF16),
        "at_enc": ([128, D], F32),
        "iota_bw": ([NBIN, D], F32),
    }


def _build_nc(t_pack):
    if t_pack in _NC_CACHE:
        return _NC_CACHE[t_pack]
    nc = bacc.Bacc(
        "TRN2",
        target_bir_lowering=False,
        debug=False,
        enable_asserts=True,
        num_devices=N_CORES,
    )
    ins = {
        name: nc.dram_tensor(name, shape, dt, kind="ExternalInput").ap()
        for name, (shape, dt) in _input_specs(t_pack).items()
    }
    outs = {"tags": nc.dram_tensor("tags", [NBIN, t_pack], I32, kind="ExternalOutput").ap()}
    with TileContext(nc) as tc:
        crf_kernel(tc, outs, ins, T=t_pack)
    nc.compile()
    _NC_CACHE[t_pack] = nc
    return nc


def _prepare(logits, sent_lengths, crf_params):
    logits = np.asarray(logits, dtype=np.float32)
    lengths = np.asarray(sent_lengths).astype(np.int64)
    t_pack, bins = pack_bins(lengths)
    consts = make_consts(crf_params, t_pack)
    in_maps = []
    for core in range(N_CORES):
        bins_core = bins[core * NBIN:(core + 1) * NBIN]
        in_maps.append(make_core_inputs(logits, bins_core, consts, t_pack))
    return t_pack, bins, in_maps


def _unpack(results, bins, lengths, t_pack):
    out = np.zeros((B, T_IN), dtype=np.int32)
    for core in range(N_CORES):
        tags = results[core]["tags"]            # [NBIN, t_pack] i32
        for bl, seqs in enumerate(bins[core * NBIN:(core + 1) * NBIN]):
            for (idx, s, L) in seqs:
                out[idx, 0:L] = tags[bl, s:s + L]
    return out


def kernel(logits, sent_lengths, crf_params):
    lengths = np.asarray(sent_lengths).astype(np.int64)
    t_pack, bins, in_maps = _prepare(logits, sent_lengths, crf_params)
    nc = _build_nc(t_pack)
    br = run_bass_kernel_spmd(nc, in_maps, core_ids=list(range(N_CORES)))
    return _unpack(br.results, bins, lengths, t_pack)


# revision 4
# speedup vs baseline: 1.0242x; 1.0005x over previous
"""nn_CRFLayer: CRF Viterbi decode on 8 Trainium2 NeuronCores — packed version.

Key idea: sent_lengths are uniform 1..512 (mean ~256), so half of all (b,t)
positions are padding. The host bin-packs the 512 sequences into 256 "bins"
(lane-slots) of T_PACK steps (multiple sequences concatenated in time, with
alpha resets at the boundaries); each core runs 32 bins. Partition layout:
p = 4*bin_local + q, q = c-quarter (4 copies of each bin), so each lane does
12 cur x 48 prev per step instead of 24 x 48 — half the element work of the
unpacked kernel, with no padded steps.

All heavy ops run on the DVE (the Pool engine's software TTs contend with
DVE for SBUF and slow both ~2x when overlapped; measured). Per step:
  add (scores = trans_q + alpha_bcast), segmented reduce_max, one fused
  scalar_tensor_tensor alpha update (maxv * not_reset + emit — handles both
  the recurrence and the boundary reset), two stream_shuffles (4-way
  alpha all-gather). Backpointer extraction (is_ge mask -> bf16 mult by
  encoded iota -> segmented reduce_min, exact first-index argmax) is
  batched x4 steps and lags the alpha chain.

Sequence boundaries in the backward chase are handled with constant-map
backpointers: a batched post-pass computes AT[bin, t] = argmax_c alpha[t]
for every t, and the backward merges AT[t-1] into the bp stream at boundary
steps, so the chase "snaps" to each sequence's last tag with a plain 2-op
per-step chain (one-hot dot-accumulate + is_equal regeneration).

All value-producing float ops are the same single fp32 adds as the
reference, so decoded tags match bitwise-exactly.
"""

import numpy as np
import ml_dtypes

import concourse.bass as bass
import concourse.mybir as mybir
from concourse.tile import TileContext

AL = mybir.AluOpType
F32 = mybir.dt.float32
BF16 = mybir.dt.bfloat16
I32 = mybir.dt.int32

D = 48
Q = 12              # cur per lane
NBIN = 32           # bins per core
N_CORES = 8
B = 512
T_IN = 512
CK = 32             # forward DMA chunk (steps)
BATCH = 16          # bp-extraction batch (steps)
CKB = 32            # backward chunk (steps)

ROT1_MASK = [(i & ~3) | ((i + 1) & 3) for i in range(32)]
ROT2_MASK = [(i & ~3) | ((i + 2) & 3) for i in range(32)]


# ---------------------------------------------------------------------------
# host-side packing
# ---------------------------------------------------------------------------

def pack_bins(lengths: np.ndarray) -> tuple[int, list[list[tuple[int, int, int]]]]:
    """FFD-pack sequences into NBIN*N_CORES bins. Returns (T_PACK, bins) where
    bins[i] = [(orig_idx, start, length), ...]."""
    nbins = NBIN * N_CORES
    order = np.argsort(-lengths, kind="stable")
    t_pack = max(512, int(np.ceil(lengths.sum() / nbins / 16) * 16))
    while True:
        fills = [0] * nbins
        bins = [[] for _ in range(nbins)]
        ok = True
        for idx in order:
            L = int(lengths[idx])
            for i in range(nbins):
                if fills[i] + L <= t_pack:
                    bins[i].append((int(idx), fills[i], L))
                    fills[i] += L
                    break
            else:
                ok = False
                break
        if ok:
            return t_pack, bins
        t_pack += 16


def make_consts(trans: np.ndarray, t_pack: int) -> dict[str, np.ndarray]:
    trans = np.asarray(trans, dtype=np.float32)
    trans_q = np.zeros((128, Q, D), dtype=np.float32)
    iota_q = np.zeros((128, Q, D), dtype=np.float32)
    for q in range(4):
        prev = (Q * q + np.arange(D)) % D          # jslot -> real j
        cur = Q * q + np.arange(Q)
        block = trans[prev][:, cur].T              # [Q, D]
        enc = np.broadcast_to((prev - 64.0)[None, :], (Q, D))
        for bl in range(NBIN):
            p = 4 * bl + q
            trans_q[p] = block
            iota_q[p] = enc
    at_enc = np.broadcast_to(
        (np.arange(D, dtype=np.float32) - 64.0)[None, :], (128, D)
    ).copy()                                        # (q,c) -> 12q+c - 64
    iota_bw = np.broadcast_to(
        (np.arange(D, dtype=np.float32) - 64.0)[None, :], (NBIN, D)
    ).copy()
    return {
        "trans_q": trans_q.reshape(128, Q * D),
        "iota_q": iota_q.reshape(128, Q * D).astype(ml_dtypes.bfloat16),
        "at_enc": at_enc,
        "iota_bw": iota_bw,
    }


def make_core_inputs(logits, bins_core, consts, t_pack) -> dict[str, np.ndarray]:
    """bins_core: 32 bins for this core, each [(orig_idx, start, L), ...]."""
    emit = np.zeros((128, t_pack, Q), dtype=np.float32)
    nr = np.ones((NBIN, t_pack), dtype=np.float32)
    bm = np.zeros((NBIN, t_pack), dtype=np.float32)
    for bl, seqs in enumerate(bins_core):
        fill = 0
        for (idx, s, L) in seqs:
            for q in range(4):
                emit[4 * bl + q, s:s + L, :] = logits[idx, :L, Q * q:Q * q + Q]
            nr[bl, s] = 0.0
            if s >= 1:
                bm[bl, s] = 1.0
            fill = s + L
        if fill < t_pack:          # junk start is a boundary too
            nr[bl, fill] = 0.0
            if fill >= 1:
                bm[bl, fill] = 1.0
    nr_il = np.repeat(nr, 4, axis=0)               # [128, T]
    nbm = 1.0 - bm
    return dict(
        consts,
        emit=np.ascontiguousarray(emit),
        nr_il=np.ascontiguousarray(nr_il),
        bm=np.ascontiguousarray(bm),
        nbm=np.ascontiguousarray(nbm),
    )


# ---------------------------------------------------------------------------
# kernel
# ---------------------------------------------------------------------------

def crf_kernel(tc: TileContext, outs, ins, T: int):
    nc = tc.nc
    TC = T // 4                     # per-(bl,tc) chunk for the AT pass

    emit_d = ins["emit"]            # [128, T, Q] dram f32
    tags_out = outs["tags"]         # [NBIN, T] dram i32

    bp_dram = nc.dram_tensor("bp_scratch", [128, T, Q], BF16, kind="Internal").ap()
    ah_dram = nc.dram_tensor("ah_scratch", [128, T, Q], F32, kind="Internal").ap()
    at_dram = nc.dram_tensor("at_scratch", [NBIN, T + 1], F32, kind="Internal").ap()

    with (
        tc.tile_pool(name="persist", bufs=1) as pp,
        tc.tile_pool(name="chunks", bufs=3) as cp,
        tc.tile_pool(name="work", bufs=1) as wp,
    ):
        # ---- persistent constants ----
        trans_q = pp.tile([128, Q, D], F32, tag="trans_q")
        nc.sync.dma_start(trans_q[:].rearrange("p a b -> p (a b)"), ins["trans_q"])
        iota_q = pp.tile([128, Q, D], BF16, tag="iota_q")
        nc.sync.dma_start(iota_q[:].rearrange("p a b -> p (a b)"), ins["iota_q"])
        at_enc = pp.tile([128, D], F32, tag="at_enc")
        nc.sync.dma_start(at_enc[:], ins["at_enc"])
        iota_bw = pp.tile([NBIN, D], F32, tag="iota_bw")
        nc.sync.dma_start(iota_bw[:], ins["iota_bw"])
        nr_il = pp.tile([128, T], F32, tag="nr_il")
        nc.sync.dma_start(nr_il[:], ins["nr_il"])
        bm = pp.tile([NBIN, T], F32, tag="bm")
        nc.sync.dma_start(bm[:], ins["bm"])
        nbm = pp.tile([NBIN, T], F32, tag="nbm")
        nc.sync.dma_start(nbm[:], ins["nbm"])

        aring = pp.tile([128, 8, D], F32, tag="aring")
        nc.vector.memset(aring[:], 0.0)

        srings = [pp.tile([128, BATCH, Q, D], F32, name=f"sring{i}", tag=f"sring{i}") for i in range(2)]
        mrings = [pp.tile([128, BATCH, Q], F32, name=f"mring{i}", tag=f"mring{i}") for i in range(2)]

        # ---- forward scan ----
        emit_tiles = {}

        def fetch_emit(t0):
            et = cp.tile([128, CK, Q], F32, name=f"emit_{t0}", tag="emit_ch")
            nc.sync.dma_start(et[:], emit_d[:, t0:t0 + CK, :])
            emit_tiles[t0] = et

        fetch_emit(0)
        for t0 in range(0, T, CK):
            if t0 + CK < T:
                fetch_emit(t0 + CK)
            emit_ch = emit_tiles.pop(t0)
            bp_ch = cp.tile([128, CK, Q], BF16, tag="bp_ch")
            for t in range(t0, t0 + CK):
                r = (t // BATCH) % 2
                k = t % BATCH
                kc = t - t0
                a_prev = aring[:, (t + 7) % 8, :]
                a_next = aring[:, t % 8, :]
                sc = srings[r]
                mv = mrings[r]
                a_b = a_prev.unsqueeze(1).broadcast_to([128, Q, D])
                nc.vector.tensor_tensor(out=sc[:, k], in0=trans_q[:], in1=a_b, op=AL.add)
                nc.vector.tensor_reduce(
                    out=mv[:, k], in_=sc[:, k], axis=mybir.AxisListType.X, op=AL.max,
                )
                # alpha = maxv * not_reset + emit  (reset -> alpha = emit)
                nc.vector.scalar_tensor_tensor(
                    out=a_next[0:128, 0:Q], in0=mv[:, k], scalar=nr_il[:, t:t + 1],
                    in1=emit_ch[:, kc, :], op0=AL.mult, op1=AL.add,
                )
                nc.vector.stream_shuffle(a_next[0:128, Q:2 * Q], a_next[0:128, 0:Q], mask=ROT1_MASK)
                nc.vector.stream_shuffle(a_next[0:128, 2 * Q:4 * Q], a_next[0:128, 0:2 * Q], mask=ROT2_MASK)
                if t % 4 == 3:
                    s4 = (t - 3) % 8
                    nc.sync.dma_start(
                        ah_dram[:, t - 3:t + 1, :], aring[:, s4:s4 + 4, 0:Q]
                    )
                if k == BATCH - 1:
                    # batched bp extraction for steps t-BATCH+1..t
                    mask4 = wp.tile([128, BATCH, Q, D], BF16, tag="mask4")
                    sc_v = sc[:].rearrange("p b c j -> p (b c) j")
                    mv_v = mv[:].rearrange("p b c -> p (b c)").unsqueeze(2)
                    nc.vector.tensor_tensor(
                        out=mask4[:].rearrange("p b c j -> p (b c) j"), in0=sc_v,
                        in1=mv_v.broadcast_to([128, BATCH * Q, D]), op=AL.is_ge,
                    )
                    nc.vector.tensor_tensor(
                        out=mask4[:], in0=mask4[:],
                        in1=iota_q[:].unsqueeze(1).broadcast_to([128, BATCH, Q, D]),
                        op=AL.mult,
                    )
                    # segmented min via bf16 TT tree (2x DVE mode; TR has no 2x)
                    fv = mask4[:].rearrange("p b c j -> p (b c) j")
                    nc.vector.tensor_tensor(
                        out=fv[:, :, 0:24], in0=fv[:, :, 0:24], in1=fv[:, :, 24:48], op=AL.min)
                    nc.vector.tensor_tensor(
                        out=fv[:, :, 0:12], in0=fv[:, :, 0:12], in1=fv[:, :, 12:24], op=AL.min)
                    nc.vector.tensor_tensor(
                        out=fv[:, :, 0:6], in0=fv[:, :, 0:6], in1=fv[:, :, 6:12], op=AL.min)
                    nc.vector.tensor_tensor(
                        out=fv[:, :, 0:3], in0=fv[:, :, 0:3], in1=fv[:, :, 3:6], op=AL.min)
                    nc.vector.tensor_tensor(
                        out=fv[:, :, 0:1], in0=fv[:, :, 0:1], in1=fv[:, :, 1:2], op=AL.min)
                    bp_out = bp_ch[:, kc - BATCH + 1:kc + 1, :].rearrange(
                        "p t c -> p (t c)").unsqueeze(2)
                    nc.vector.tensor_tensor(
                        out=bp_out, in0=fv[:, :, 0:1], in1=fv[:, :, 2:3], op=AL.min)
            nc.sync.dma_start(bp_dram[:, t0:t0 + CK, :], bp_ch[:])

        # ---- AT pass: AT[bin, t] = enc(first-argmax_c alpha[bin, t, :]) ----
        # atile partitions = (tc, b): lane tc*32+b covers t in [tc*TC, (tc+1)*TC)
        ah_v4 = ah_dram.rearrange("(b q) t c -> b q t c", q=4)
        with tc.tile_pool(name="atpool", bufs=1) as ap:
            atile = ap.tile([128, TC, 4, Q], F32, tag="atile")
            for tc4 in range(4):
                for q in range(4):
                    nc.sync.dma_start(
                        atile[tc4 * NBIN:(tc4 + 1) * NBIN, :, q, :],
                        ah_v4[:, q, tc4 * TC:(tc4 + 1) * TC, :],
                    )
            atmax = ap.tile([128, TC], F32, tag="atmax")
            nc.vector.tensor_reduce(
                out=atmax[:], in_=atile[:], axis=mybir.AxisListType.XY, op=AL.max,
            )
            atmask = ap.tile([128, TC, D], BF16, tag="atmask")
            nc.vector.tensor_tensor(
                out=atmask[:], in0=atile[:].rearrange("p t q c -> p t (q c)"),
                in1=atmax[:].unsqueeze(2).broadcast_to([128, TC, D]), op=AL.is_ge,
            )
            atf = ap.tile([128, TC, D], BF16, tag="atf")
            nc.vector.tensor_tensor(
                out=atf[:], in0=atmask[:],
                in1=at_enc[:].unsqueeze(1).broadcast_to([128, TC, D]),
                op=AL.mult,
            )
            at_all = ap.tile([128, TC], F32, tag="at_all")
            nc.vector.tensor_reduce(
                out=at_all[:], in_=atf[:], axis=mybir.AxisListType.X, op=AL.min,
            )
            # at_dram[bin, 1 + t] = AT[bin, t]
            for tc4 in range(4):
                nc.sync.dma_start(
                    at_dram[:, 1 + tc4 * TC:1 + (tc4 + 1) * TC],
                    at_all[tc4 * NBIN:(tc4 + 1) * NBIN, :],
                )

        # ---- backward chase ----
        h = pp.tile([NBIN, D], F32, tag="h")
        tagsq = pp.tile([NBIN, T], F32, tag="tagsq")
        junk = pp.tile([NBIN, D], F32, tag="junk")
        at_last = pp.tile([NBIN, 1], F32, tag="at_last")
        nc.sync.dma_start(at_last[:], at_dram[:, T:T + 1])
        nc.vector.tensor_copy(out=tagsq[:, T - 1:T], in_=at_last[:])
        nc.vector.tensor_scalar(
            out=h[:], in0=iota_bw[:], scalar1=at_last[:, 0:1], scalar2=None,
            op0=AL.is_equal,
        )

        bp_v = bp_dram.rearrange("(b q) t c -> b q t c", q=4)
        bwp_ctx = tc.tile_pool(name="bwp", bufs=2)
        bwp = bwp_ctx.__enter__()
        for c0 in range(T - 1, 0, -CKB):
            ckb = min(CKB, c0)      # bp indices c0, c0-1, ..., c0-ckb+1 (>=1)
            lo = c0 - ckb + 1
            bpb = bwp.tile([NBIN, CKB, D], BF16, tag="bpb")
            for q in range(4):
                nc.sync.dma_start(
                    bpb[:, 0:ckb, Q * q:Q * (q + 1)], bp_v[:, q, lo:c0 + 1, :]
                )
            atp = bwp.tile([NBIN, CKB], F32, tag="atp")
            nc.sync.dma_start(atp[:, 0:ckb], at_dram[:, lo:c0 + 1])
            # merge: bpb2 = bpb * nbm + AT[t-1] * bm   (boundary const-maps)
            bpb2 = bwp.tile([NBIN, CKB, D], F32, tag="bpb2")
            nc.vector.tensor_tensor(
                out=bpb2[:, 0:ckb, :], in0=bpb[:, 0:ckb, :],
                in1=nbm[:, lo:c0 + 1].unsqueeze(2).broadcast_to([NBIN, ckb, D]),
                op=AL.mult,
            )
            atpm = bwp.tile([NBIN, CKB], F32, tag="atpm")
            nc.vector.tensor_tensor(
                out=atpm[:, 0:ckb], in0=atp[:, 0:ckb],
                in1=bm[:, lo:c0 + 1], op=AL.mult,
            )
            nc.vector.tensor_tensor(
                out=bpb2[:, 0:ckb, :], in0=bpb2[:, 0:ckb, :],
                in1=atpm[:, 0:ckb].unsqueeze(2).broadcast_to([NBIN, ckb, D]), op=AL.add,
            )
            for t in range(c0, lo - 1, -1):
                kk = t - lo
                nc.vector.scalar_tensor_tensor(
                    out=junk[:], in0=bpb2[:, kk, :], scalar=1.0, in1=h[:],
                    op0=AL.mult, op1=AL.mult, accum_out=tagsq[:, t - 1:t],
                )
                if t > 1:
                    nc.vector.tensor_tensor(
                        out=h[:], in0=iota_bw[:],
                        in1=tagsq[:, t - 1:t].broadcast_to([NBIN, D]),
                        op=AL.is_equal,
                    )

        bwp_ctx.__exit__(None, None, None)

        # ---- decode (+64) + cast + store ----
        tags_f = pp.tile([NBIN, T], F32, tag="tags_f")
        nc.vector.tensor_scalar(
            out=tags_f[:], in0=tagsq[:], scalar1=64.0, scalar2=None, op0=AL.add,
        )
        tags_i = pp.tile([NBIN, T], I32, tag="tags_i")
        nc.vector.tensor_copy(out=tags_i[:], in_=tags_f[:])
        nc.sync.dma_start(tags_out, tags_i[:])


# ---------------------------------------------------------------------------
# self-contained harness
# ---------------------------------------------------------------------------
import concourse.bacc as bacc
from concourse.bass_utils import run_bass_kernel_spmd

_NC_CACHE: dict[int, object] = {}


def _input_specs(t_pack):
    return {
        "emit": ([128, t_pack, Q], F32),
        "nr_il": ([128, t_pack], F32),
        "bm": ([NBIN, t_pack], F32),
        "nbm": ([NBIN, t_pack], F32),
        "trans_q": ([128, Q * D], F32),
        "iota_q": ([128, Q * D], BF16),
        "at_enc": ([128, D], F32),
        "iota_bw": ([NBIN, D], F32),
    }


def _build_nc(t_pack):
    if t_pack in _NC_CACHE:
        return _NC_CACHE[t_pack]
    nc = bacc.Bacc(
        "TRN2",
        target_bir_lowering=False,
        debug=False,
        enable_asserts=True,
        num_devices=N_CORES,
    )
    ins = {
        name: nc.dram_tensor(name, shape, dt, kind="ExternalInput").ap()
        for name, (shape, dt) in _input_specs(t_pack).items()
    }
    outs = {"tags": nc.dram_tensor("tags", [NBIN, t_pack], I32, kind="ExternalOutput").ap()}
    with TileContext(nc) as tc:
        crf_kernel(tc, outs, ins, T=t_pack)
    nc.compile()
    _NC_CACHE[t_pack] = nc
    return nc


def _prepare(logits, sent_lengths, crf_params):
    logits = np.asarray(logits, dtype=np.float32)
    lengths = np.asarray(sent_lengths).astype(np.int64)
    t_pack, bins = pack_bins(lengths)
    consts = make_consts(crf_params, t_pack)
    in_maps = []
    for core in range(N_CORES):
        bins_core = bins[core * NBIN:(core + 1) * NBIN]
        in_maps.append(make_core_inputs(logits, bins_core, consts, t_pack))
    return t_pack, bins, in_maps


def _unpack(results, bins, lengths, t_pack):
    out = np.zeros((B, T_IN), dtype=np.int32)
    for core in range(N_CORES):
        tags = results[core]["tags"]            # [NBIN, t_pack] i32
        for bl, seqs in enumerate(bins[core * NBIN:(core + 1) * NBIN]):
            for (idx, s, L) in seqs:
                out[idx, 0:L] = tags[bl, s:s + L]
    return out


def kernel(logits, sent_lengths, crf_params):
    lengths = np.asarray(sent_lengths).astype(np.int64)
    t_pack, bins, in_maps = _prepare(logits, sent_lengths, crf_params)
    nc = _build_nc(t_pack)
    br = run_bass_kernel_spmd(nc, in_maps, core_ids=list(range(N_CORES)))
    return _unpack(br.results, bins, lengths, t_pack)


# revision 5
# speedup vs baseline: 1.0366x; 1.0121x over previous
"""nn_CRFLayer: CRF Viterbi decode on 8 Trainium2 NeuronCores — packed version.

Key idea: sent_lengths are uniform 1..512 (mean ~256), so half of all (b,t)
positions are padding. The host bin-packs the 512 sequences into 256 "bins"
(lane-slots) of T_PACK steps (multiple sequences concatenated in time, with
alpha resets at the boundaries); each core runs 32 bins. Partition layout:
p = 4*bin_local + q, q = c-quarter (4 copies of each bin), so each lane does
12 cur x 48 prev per step instead of 24 x 48 — half the element work of the
unpacked kernel, with no padded steps.

All heavy ops run on the DVE (the Pool engine's software TTs contend with
DVE for SBUF and slow both ~2x when overlapped; measured). Per step:
  add (scores = trans_q + alpha_bcast), segmented reduce_max, one fused
  scalar_tensor_tensor alpha update (maxv * not_reset + emit — handles both
  the recurrence and the boundary reset), two stream_shuffles (4-way
  alpha all-gather). Backpointer extraction (is_ge mask -> bf16 mult by
  encoded iota -> segmented reduce_min, exact first-index argmax) is
  batched x4 steps and lags the alpha chain.

Sequence boundaries in the backward chase are handled with constant-map
backpointers: a batched post-pass computes AT[bin, t] = argmax_c alpha[t]
for every t, and the backward merges AT[t-1] into the bp stream at boundary
steps, so the chase "snaps" to each sequence's last tag with a plain 2-op
per-step chain (one-hot dot-accumulate + is_equal regeneration).

All value-producing float ops are the same single fp32 adds as the
reference, so decoded tags match bitwise-exactly.
"""

import numpy as np
import ml_dtypes

import concourse.bass as bass
import concourse.mybir as mybir
from concourse.tile import TileContext

AL = mybir.AluOpType
F32 = mybir.dt.float32
BF16 = mybir.dt.bfloat16
I32 = mybir.dt.int32

D = 48
Q = 12              # cur per lane
NBIN = 32           # bins per core
N_CORES = 8
B = 512
T_IN = 512
CK = 32             # forward DMA chunk (steps)
BATCH = 16          # bp-extraction batch (steps)
CKB = 32            # backward chunk (steps)

ROT1_MASK = [(i & ~3) | ((i + 1) & 3) for i in range(32)]
ROT2_MASK = [(i & ~3) | ((i + 2) & 3) for i in range(32)]


# ---------------------------------------------------------------------------
# host-side packing
# ---------------------------------------------------------------------------

def pack_bins(lengths: np.ndarray) -> tuple[int, list[list[tuple[int, int, int]]]]:
    """FFD-pack sequences into NBIN*N_CORES bins. Returns (T_PACK, bins) where
    bins[i] = [(orig_idx, start, length), ...]."""
    nbins = NBIN * N_CORES
    order = np.argsort(-lengths, kind="stable")
    t_pack = max(512, int(np.ceil(lengths.sum() / nbins / 16) * 16))
    while True:
        fills = [0] * nbins
        bins = [[] for _ in range(nbins)]
        ok = True
        for idx in order:
            L = int(lengths[idx])
            for i in range(nbins):
                if fills[i] + L <= t_pack:
                    bins[i].append((int(idx), fills[i], L))
                    fills[i] += L
                    break
            else:
                ok = False
                break
        if ok:
            return t_pack, bins
        t_pack += 16


def make_consts(trans: np.ndarray, t_pack: int) -> dict[str, np.ndarray]:
    trans = np.asarray(trans, dtype=np.float32)
    trans_q = np.zeros((128, Q, D), dtype=np.float32)
    iota_q = np.zeros((128, Q, D), dtype=np.float32)
    for q in range(4):
        prev = (Q * q + np.arange(D)) % D          # jslot -> real j
        cur = Q * q + np.arange(Q)
        block = trans[prev][:, cur].T              # [Q, D]
        enc = np.broadcast_to((prev - 64.0)[None, :], (Q, D))
        for bl in range(NBIN):
            p = 4 * bl + q
            trans_q[p] = block
            iota_q[p] = enc
    at_enc = np.broadcast_to(
        (np.arange(D, dtype=np.float32) - 64.0)[None, :], (128, D)
    ).copy()                                        # (q,c) -> 12q+c - 64
    iota_bw = np.broadcast_to(
        (np.arange(D, dtype=np.float32) - 64.0)[None, :], (NBIN, D)
    ).copy()
    return {
        "trans_q": trans_q.reshape(128, Q * D),
        "iota_q": iota_q.reshape(128, Q * D).astype(ml_dtypes.bfloat16),
        "at_enc": at_enc,
        "iota_bw": iota_bw,
    }


def make_core_inputs(logits, bins_core, consts, t_pack) -> dict[str, np.ndarray]:
    """bins_core: 32 bins for this core, each [(orig_idx, start, L), ...]."""
    emit = np.zeros((128, t_pack, Q), dtype=np.float32)
    nr = np.ones((NBIN, t_pack), dtype=np.float32)
    bm = np.zeros((NBIN, t_pack), dtype=np.float32)
    for bl, seqs in enumerate(bins_core):
        fill = 0
        for (idx, s, L) in seqs:
            for q in range(4):
                emit[4 * bl + q, s:s + L, :] = logits[idx, :L, Q * q:Q * q + Q]
            nr[bl, s] = 0.0
            if s >= 1:
                bm[bl, s] = 1.0
            fill = s + L
        if fill < t_pack:          # junk start is a boundary too
            nr[bl, fill] = 0.0
            if fill >= 1:
                bm[bl, fill] = 1.0
    nr_il = np.repeat(nr, 4, axis=0)               # [128, T]
    nbm = 1.0 - bm
    return dict(
        consts,
        emit=np.ascontiguousarray(emit),
        nr_il=np.ascontiguousarray(nr_il),
        bm=np.ascontiguousarray(bm),
        nbm=np.ascontiguousarray(nbm),
    )


# ---------------------------------------------------------------------------
# kernel
# ---------------------------------------------------------------------------

def crf_kernel(tc: TileContext, outs, ins, T: int):
    nc = tc.nc
    TC = T // 4                     # per-(bl,tc) chunk for the AT pass

    emit_d = ins["emit"]            # [128, T, Q] dram f32
    tags_out = outs["tags"]         # [NBIN, T] dram i32

    bp_dram = nc.dram_tensor("bp_scratch", [128, T, Q], BF16, kind="Internal").ap()
    ah_dram = nc.dram_tensor("ah_scratch", [128, T, Q], F32, kind="Internal").ap()
    at_dram = nc.dram_tensor("at_scratch", [NBIN, T + 1], F32, kind="Internal").ap()

    with (
        tc.tile_pool(name="persist", bufs=1) as pp,
        tc.tile_pool(name="chunks", bufs=3) as cp,
        tc.tile_pool(name="work", bufs=1) as wp,
    ):
        # ---- persistent constants ----
        trans_q = pp.tile([128, Q, D], F32, tag="trans_q")
        nc.sync.dma_start(trans_q[:].rearrange("p a b -> p (a b)"), ins["trans_q"])
        iota_q = pp.tile([128, Q, D], BF16, tag="iota_q")
        nc.sync.dma_start(iota_q[:].rearrange("p a b -> p (a b)"), ins["iota_q"])
        at_enc = pp.tile([128, D], F32, tag="at_enc")
        nc.sync.dma_start(at_enc[:], ins["at_enc"])
        iota_bw = pp.tile([NBIN, D], F32, tag="iota_bw")
        nc.sync.dma_start(iota_bw[:], ins["iota_bw"])
        nr_il = pp.tile([128, T], F32, tag="nr_il")
        nc.sync.dma_start(nr_il[:], ins["nr_il"])
        bm = pp.tile([NBIN, T], F32, tag="bm")
        nc.sync.dma_start(bm[:], ins["bm"])
        nbm = pp.tile([NBIN, T], F32, tag="nbm")
        nc.sync.dma_start(nbm[:], ins["nbm"])

        aring = pp.tile([128, 8, D], F32, tag="aring")
        nc.vector.memset(aring[:], 0.0)

        srings = [pp.tile([128, BATCH, Q, D], F32, name=f"sring{i}", tag=f"sring{i}") for i in range(2)]
        mrings = [pp.tile([128, BATCH, Q], F32, name=f"mring{i}", tag=f"mring{i}") for i in range(2)]

        # ---- forward scan ----
        emit_tiles = {}

        def fetch_emit(t0):
            et = cp.tile([128, CK, Q], F32, name=f"emit_{t0}", tag="emit_ch")
            nc.sync.dma_start(et[:], emit_d[:, t0:t0 + CK, :])
            emit_tiles[t0] = et

        fetch_emit(0)
        for t0 in range(0, T, CK):
            if t0 + CK < T:
                fetch_emit(t0 + CK)
            emit_ch = emit_tiles.pop(t0)
            bp_ch = cp.tile([128, CK, Q], BF16, tag="bp_ch")
            for t in range(t0, t0 + CK):
                r = (t // BATCH) % 2
                k = t % BATCH
                kc = t - t0
                a_prev = aring[:, (t + 7) % 8, :]
                a_next = aring[:, t % 8, :]
                sc = srings[r]
                mv = mrings[r]
                a_b = a_prev.unsqueeze(1).broadcast_to([128, Q, D])
                nc.vector.tensor_tensor(out=sc[:, k], in0=trans_q[:], in1=a_b, op=AL.add)
                nc.vector.tensor_reduce(
                    out=mv[:, k], in_=sc[:, k], axis=mybir.AxisListType.X, op=AL.max,
                )
                # alpha = maxv * not_reset + emit  (reset -> alpha = emit)
                nc.vector.scalar_tensor_tensor(
                    out=a_next[0:128, 0:Q], in0=mv[:, k], scalar=nr_il[:, t:t + 1],
                    in1=emit_ch[:, kc, :], op0=AL.mult, op1=AL.add,
                )
                nc.vector.stream_shuffle(a_next[0:128, Q:2 * Q], a_next[0:128, 0:Q], mask=ROT1_MASK)
                nc.vector.stream_shuffle(a_next[0:128, 2 * Q:4 * Q], a_next[0:128, 0:2 * Q], mask=ROT2_MASK)
                if t % 4 == 3:
                    s4 = (t - 3) % 8
                    nc.sync.dma_start(
                        ah_dram[:, t - 3:t + 1, :], aring[:, s4:s4 + 4, 0:Q]
                    )
                if k == BATCH - 1:
                    # batched bp extraction for steps t-BATCH+1..t
                    mask4 = wp.tile([128, BATCH, Q, D], BF16, tag="mask4")
                    sc_v = sc[:].rearrange("p b c j -> p (b c) j")
                    mv_v = mv[:].rearrange("p b c -> p (b c)").unsqueeze(2)
                    nc.vector.tensor_tensor(
                        out=mask4[:].rearrange("p b c j -> p (b c) j"), in0=sc_v,
                        in1=mv_v.broadcast_to([128, BATCH * Q, D]), op=AL.is_ge,
                    )
                    nc.vector.tensor_tensor(
                        out=mask4[:], in0=mask4[:],
                        in1=iota_q[:].unsqueeze(1).broadcast_to([128, BATCH, Q, D]),
                        op=AL.mult,
                    )
                    # segmented min via bf16 TT tree (2x DVE mode; TR has no 2x)
                    fv = mask4[:].rearrange("p b c j -> p (b c) j")
                    nc.vector.tensor_tensor(
                        out=fv[:, :, 0:24], in0=fv[:, :, 0:24], in1=fv[:, :, 24:48], op=AL.min)
                    nc.vector.tensor_tensor(
                        out=fv[:, :, 0:12], in0=fv[:, :, 0:12], in1=fv[:, :, 12:24], op=AL.min)
                    nc.vector.tensor_tensor(
                        out=fv[:, :, 0:6], in0=fv[:, :, 0:6], in1=fv[:, :, 6:12], op=AL.min)
                    nc.vector.tensor_tensor(
                        out=fv[:, :, 0:3], in0=fv[:, :, 0:3], in1=fv[:, :, 3:6], op=AL.min)
                    # final 3 -> 1 via one contiguous TR (strided 1-elem TTs are slow)
                    nc.vector.tensor_reduce(
                        out=bp_ch[:, kc - BATCH + 1:kc + 1, :], in_=fv[:, :, 0:3],
                        axis=mybir.AxisListType.X, op=AL.min,
                    )
            nc.sync.dma_start(bp_dram[:, t0:t0 + CK, :], bp_ch[:])

        # ---- AT pass: AT[bin, t] = enc(first-argmax_c alpha[bin, t, :]) ----
        # atile partitions = (tc, b): lane tc*32+b covers t in [tc*TC, (tc+1)*TC)
        ah_v4 = ah_dram.rearrange("(b q) t c -> b q t c", q=4)
        with tc.tile_pool(name="atpool", bufs=1) as ap:
            atile = ap.tile([128, TC, 4, Q], F32, tag="atile")
            for tc4 in range(4):
                for q in range(4):
                    nc.sync.dma_start(
                        atile[tc4 * NBIN:(tc4 + 1) * NBIN, :, q, :],
                        ah_v4[:, q, tc4 * TC:(tc4 + 1) * TC, :],
                    )
            atmax = ap.tile([128, TC], F32, tag="atmax")
            nc.vector.tensor_reduce(
                out=atmax[:], in_=atile[:], axis=mybir.AxisListType.XY, op=AL.max,
            )
            atmask = ap.tile([128, TC, D], BF16, tag="atmask")
            nc.vector.tensor_tensor(
                out=atmask[:], in0=atile[:].rearrange("p t q c -> p t (q c)"),
                in1=atmax[:].unsqueeze(2).broadcast_to([128, TC, D]), op=AL.is_ge,
            )
            atf = ap.tile([128, TC, D], BF16, tag="atf")
            nc.vector.tensor_tensor(
                out=atf[:], in0=atmask[:],
                in1=at_enc[:].unsqueeze(1).broadcast_to([128, TC, D]),
                op=AL.mult,
            )
            at_all = ap.tile([128, TC], F32, tag="at_all")
            nc.vector.tensor_reduce(
                out=at_all[:], in_=atf[:], axis=mybir.AxisListType.X, op=AL.min,
            )
            # at_dram[bin, 1 + t] = AT[bin, t]
            for tc4 in range(4):
                nc.sync.dma_start(
                    at_dram[:, 1 + tc4 * TC:1 + (tc4 + 1) * TC],
                    at_all[tc4 * NBIN:(tc4 + 1) * NBIN, :],
                )

        # ---- backward chase ----
        h = pp.tile([NBIN, D], F32, tag="h")
        tagsq = pp.tile([NBIN, T], F32, tag="tagsq")
        junk = pp.tile([NBIN, D], F32, tag="junk")
        at_last = pp.tile([NBIN, 1], F32, tag="at_last")
        nc.sync.dma_start(at_last[:], at_dram[:, T:T + 1])
        nc.vector.tensor_copy(out=tagsq[:, T - 1:T], in_=at_last[:])
        nc.vector.tensor_scalar(
            out=h[:], in0=iota_bw[:], scalar1=at_last[:, 0:1], scalar2=None,
            op0=AL.is_equal,
        )

        bp_v = bp_dram.rearrange("(b q) t c -> b q t c", q=4)
        bwp_ctx = tc.tile_pool(name="bwp", bufs=2)
        bwp = bwp_ctx.__enter__()
        for c0 in range(T - 1, 0, -CKB):
            ckb = min(CKB, c0)      # bp indices c0, c0-1, ..., c0-ckb+1 (>=1)
            lo = c0 - ckb + 1
            bpb = bwp.tile([NBIN, CKB, D], BF16, tag="bpb")
            for q in range(4):
                nc.sync.dma_start(
                    bpb[:, 0:ckb, Q * q:Q * (q + 1)], bp_v[:, q, lo:c0 + 1, :]
                )
            atp = bwp.tile([NBIN, CKB], F32, tag="atp")
            nc.sync.dma_start(atp[:, 0:ckb], at_dram[:, lo:c0 + 1])
            # merge: bpb2 = bpb * nbm + AT[t-1] * bm   (boundary const-maps)
            bpb2 = bwp.tile([NBIN, CKB, D], F32, tag="bpb2")
            nc.vector.tensor_tensor(
                out=bpb2[:, 0:ckb, :], in0=bpb[:, 0:ckb, :],
                in1=nbm[:, lo:c0 + 1].unsqueeze(2).broadcast_to([NBIN, ckb, D]),
                op=AL.mult,
            )
            atpm = bwp.tile([NBIN, CKB], F32, tag="atpm")
            nc.vector.tensor_tensor(
                out=atpm[:, 0:ckb], in0=atp[:, 0:ckb],
                in1=bm[:, lo:c0 + 1], op=AL.mult,
            )
            nc.vector.tensor_tensor(
                out=bpb2[:, 0:ckb, :], in0=bpb2[:, 0:ckb, :],
                in1=atpm[:, 0:ckb].unsqueeze(2).broadcast_to([NBIN, ckb, D]), op=AL.add,
            )
            for t in range(c0, lo - 1, -1):
                kk = t - lo
                nc.vector.scalar_tensor_tensor(
                    out=junk[:], in0=bpb2[:, kk, :], scalar=1.0, in1=h[:],
                    op0=AL.mult, op1=AL.mult, accum_out=tagsq[:, t - 1:t],
                )
                if t > 1:
                    nc.vector.tensor_tensor(
                        out=h[:], in0=iota_bw[:],
                        in1=tagsq[:, t - 1:t].broadcast_to([NBIN, D]),
                        op=AL.is_equal,
                    )

        bwp_ctx.__exit__(None, None, None)

        # ---- decode (+64) + cast + store ----
        tags_f = pp.tile([NBIN, T], F32, tag="tags_f")
        nc.vector.tensor_scalar(
            out=tags_f[:], in0=tagsq[:], scalar1=64.0, scalar2=None, op0=AL.add,
        )
        tags_i = pp.tile([NBIN, T], I32, tag="tags_i")
        nc.vector.tensor_copy(out=tags_i[:], in_=tags_f[:])
        nc.sync.dma_start(tags_out, tags_i[:])


# ---------------------------------------------------------------------------
# self-contained harness
# ---------------------------------------------------------------------------
import concourse.bacc as bacc
from concourse.bass_utils import run_bass_kernel_spmd

_NC_CACHE: dict[int, object] = {}


def _input_specs(t_pack):
    return {
        "emit": ([128, t_pack, Q], F32),
        "nr_il": ([128, t_pack], F32),
        "bm": ([NBIN, t_pack], F32),
        "nbm": ([NBIN, t_pack], F32),
        "trans_q": ([128, Q * D], F32),
        "iota_q": ([128, Q * D], BF16),
        "at_enc": ([128, D], F32),
        "iota_bw": ([NBIN, D], F32),
    }


def _build_nc(t_pack):
    if t_pack in _NC_CACHE:
        return _NC_CACHE[t_pack]
    nc = bacc.Bacc(
        "TRN2",
        target_bir_lowering=False,
        debug=False,
        enable_asserts=True,
        num_devices=N_CORES,
    )
    ins = {
        name: nc.dram_tensor(name, shape, dt, kind="ExternalInput").ap()
        for name, (shape, dt) in _input_specs(t_pack).items()
    }
    outs = {"tags": nc.dram_tensor("tags", [NBIN, t_pack], I32, kind="ExternalOutput").ap()}
    with TileContext(nc) as tc:
        crf_kernel(tc, outs, ins, T=t_pack)
    nc.compile()
    _NC_CACHE[t_pack] = nc
    return nc


def _prepare(logits, sent_lengths, crf_params):
    logits = np.asarray(logits, dtype=np.float32)
    lengths = np.asarray(sent_lengths).astype(np.int64)
    t_pack, bins = pack_bins(lengths)
    consts = make_consts(crf_params, t_pack)
    in_maps = []
    for core in range(N_CORES):
        bins_core = bins[core * NBIN:(core + 1) * NBIN]
        in_maps.append(make_core_inputs(logits, bins_core, consts, t_pack))
    return t_pack, bins, in_maps


def _unpack(results, bins, lengths, t_pack):
    out = np.zeros((B, T_IN), dtype=np.int32)
    for core in range(N_CORES):
        tags = results[core]["tags"]            # [NBIN, t_pack] i32
        for bl, seqs in enumerate(bins[core * NBIN:(core + 1) * NBIN]):
            for (idx, s, L) in seqs:
                out[idx, 0:L] = tags[bl, s:s + L]
    return out


def kernel(logits, sent_lengths, crf_params):
    lengths = np.asarray(sent_lengths).astype(np.int64)
    t_pack, bins, in_maps = _prepare(logits, sent_lengths, crf_params)
    nc = _build_nc(t_pack)
    br = run_bass_kernel_spmd(nc, in_maps, core_ids=list(range(N_CORES)))
    return _unpack(br.results, bins, lengths, t_pack)
